# revision 32
# baseline (speedup 1.0000x reference)
"""CRF Viterbi decode kernel for Trainium2 (Bass), data-parallel over batch.

Problem shapes (hardcoded): X [32,128,10000] f32 one-hot, t_feats [48,48],
e_feats [48,10000].  Output Y_hat [32,128,48] f32 one-hot.

Sharding: batch 32 -> 8 cores x 4.  t_feats / e_feats replicated.

Per-core pipeline (4 batch elems, labels on 48 partitions throughout):
  1. emissions em[l, (b,t)] = e_feats @ X^T: the host pre-transposes the
     one-hot X into an fp8 [v%128, kpair, j, (b t)] blob (0/1 exact in
     fp8) and splits e_feats^T into 4 exact fp8 terms (term 0 unscaled
     into PSUM bank A; terms 1-3 in 2^8-scaled space into bank B;
     combined as A + 2^-8 B).  Emissions are then 160 DoubleRow fp8
     matmuls (2 v-chunks per pass, 0.5 cyc/row) chasing the sliced X
     DMA -- no on-device transposes or staging.
  2. forward Viterbi: per pair of batch elems a PSUM-resident score
     tile psc[j, (sub i)] = t[i,j] + delta[i], updated per step with
     two 48-col f32 broadcast matmuls per elem: on-path bcast(m_{p-1})
     and off-path bcast(demf[p-3] - m_{p-2}) (2-step slack, built on
     the idle Pool engine).  One merged DVE tensor_reduce over both
     pairs' banks produces m_p for all 4 elems.  Periodic staggered
     full refreshes bound fp drift (exact f32 elsewhere).
  3. backward "end-anytime" scan, same structure with t^T, 56-stride
     segments whose permanent zero columns implement beta = max(0, .).
     Runs anti-phased with the forward scan: per ~1us step the two
     chains interleave PE broadcast clusters with DVE reduces.
  4. combine: y_n = onehot(argmax_j delta_n(j)+beta_n(j)) * [n <=
     end_n] via per-elem PE transposes + DVE max/max_index (first-index
     tie semantics match the reference argmax); the middle score
     columns are pre-summed during the scan loop.  Output is written
     label-minor [T, B, L] and unscrambled on the host.

Numerics notes for this stack: float32r matmuls round their inputs on
the NEFF/ucode path (CoreSim does not model it), so every scan matmul
stays plain f32; fp8/bf16 matmul inputs are exact by construction.
GPSIMD cannot touch PSUM, scalar_tensor_tensor does not exist on Pool,
and partition_all_reduce does not compile -- hence the DVE-only
reduces and Pool doing only small SBUF tensor_tensor work.
"""

import os
import sys

import numpy as np

for _p in ("/opt/trn_rl_repo",):
    if _p not in sys.path and os.path.isdir(_p):
        sys.path.insert(0, _p)

import concourse.bass as bass
import concourse.tile as tile
from concourse import mybir
from concourse.bass_utils import run_bass_kernel_spmd

F32 = mybir.dt.float32
F32R = mybir.dt.float32r
BF16 = mybir.dt.bfloat16
FP8 = mybir.dt.float8e4
AL = mybir.AluOpType
AX = mybir.AxisListType

B, T, V, L = 32, 128, 10000, 48
NCORES = 8
BLOC = B // NCORES          # 4 batch elems per core
NK = 80                     # V chunks of 128 (tail zero-padded; even for DoubleRow)
KP = NK // 2                # DoubleRow processes 2 chunks per matmul
NTERM = 4                   # fp8 e-term count (exact split of e_feats)
BT = BLOC * T               # 512 moving columns, b-major
NEG = -1.0e30
P2 = 2 * L                  # 96

# consts layout [128, CW] f32:
#  cols 0:48     tfS   [96,48]  vstack(t, t)        (fwd refresh lhsT)
#  cols 48:96    tbr   [48,48]  t.T                 (bwd refresh lhsT)
#  col  96       d0stack [96,1]
#  col  97       d0col   [48,1]
#  cols 98:227   iota129 [48,129]
#  cols 227:275  iota48  [128,48]
CW = 275

# fwd refresh schedule: always at p==2 (clears the +-1e30 d0 arithmetic),
# then every 16 steps staggered per pair.  bwd staggered likewise on t.
RF = 16


def _fwd_refresh(p, pair):
    if p == 2:
        return True
    return p > 2 and (p - 2 - 8 * pair) % RF == 0


def _bwd_refresh(t, pair):
    if t == 0:
        return True
    return (t - 6 - 8 * pair) % RF == 0


def build_nc():
    nc = bass.Bass()

    x = nc.declare_dram_parameter("x", [128, KP, 2, BT], FP8, isOutput=False)
    eT = nc.declare_dram_parameter("eT", [128, KP, NTERM, 2, L], FP8, isOutput=False)
    consts = nc.declare_dram_parameter("consts", [128, CW], F32, isOutput=False)
    idb = nc.declare_dram_parameter("idb", [128, 128], F32, isOutput=False)
    y = nc.declare_dram_parameter("y", [T, BLOC, L], F32, isOutput=True)

    with tile.TileContext(nc) as tc:
        from contextlib import ExitStack

        with ExitStack() as ctx:
            cons = ctx.enter_context(tc.tile_pool(name="cons", bufs=1))
            pers = ctx.enter_context(tc.tile_pool(name="pers", bufs=1))
            ppem = ctx.enter_context(tc.tile_pool(name="ppem", bufs=1, space="PSUM"))
            ppfw = ctx.enter_context(tc.tile_pool(name="ppfw", bufs=1, space="PSUM"))
            ppbw = ctx.enter_context(tc.tile_pool(name="ppbw", bufs=1, space="PSUM"))
            ppcb = ctx.enter_context(tc.tile_pool(name="ppcb", bufs=1, space="PSUM"))

            # ---- constants ----
            cons_sb = cons.tile([128, CW], F32)
            nc.sync.dma_start(out=cons_sb, in_=consts[:, :])
            d0col = cons_sb[0:L, 97:98]
            io129 = cons_sb[0:L, 98:227]
            io48 = cons_sb[:, 227:275]

            tfr = cons_sb[0:P2, 0:L]
            tbr = cons_sb[0:L, L : 2 * L]
            d0r = cons_sb[0:P2, 96:97]

            idb_sb = cons.tile([128, 128], F32)
            nc.sync.dma_start(out=idb_sb, in_=idb[:, :])
            id96 = idb_sb[0:P2, 0:P2]
            id48 = idb_sb[0:L, 0:L]

            # ---- persistent state ----
            em48 = pers.tile([L, BLOC, T], F32, name="em48")
            demf = pers.tile([L, BLOC, T - 1], F32, name="demf")
            ndemf = pers.tile([L, BLOC, T - 1], F32, name="ndemf")
            mslab = pers.tile([L, BLOC, T + 1], F32, name="mslab")
            bslab = pers.tile([L, BLOC, T + 2], F32, name="bslab")
            ndm = pers.tile([L, 2, BLOC], F32, name="ndm")
            nbt = pers.tile([L, 2, BLOC], F32, name="nbt")
            dsl = pers.tile([L, BLOC, T + 1], F32, name="dsl")
            fsl = pers.tile([L, BLOC, T + 1], F32, name="fsl")
            fi8 = pers.tile([T, BLOC, 8], mybir.dt.uint32, name="fi8")
            fm8 = pers.tile([T, BLOC, 8], F32, name="fm8")
            fidx = pers.tile([T, BLOC], F32, name="fidx")
            nm = pers.tile([T, BLOC], F32, name="nm")
            nmb = pers.tile([BLOC, T + 1], F32, name="nmb")
            en8 = pers.tile([BLOC, 8], F32, name="en8")
            eni8 = pers.tile([BLOC, 8], mybir.dt.uint32, name="eni8")
            endf = pers.tile([BLOC, 1], F32, name="endf")
            act = pers.tile([BLOC, T + 1], F32, name="act")
            actT = pers.tile([T, BLOC], F32, name="actT")
            ybig = pers.tile([T, BLOC, L], F32, name="ybig")

            pscA = ppfw.tile([L, 2, 512], F32, name="pscA")
            psbA = ppbw.tile([L, 2, 512], F32, name="psbA")
            pem = ppem.tile([L, 2, 512], F32, name="pem")
            # bwd uses 49-wide segments; cols 48/97 stay 0 forever (the
            # max-with-zero floor).  beta_T = 0 (bslab col T).
            nc.vector.memset(psbA[:, :, 48:105:56], 0.0)
            nc.vector.memset(bslab[:, :, T : T + 1], 0.0)

            def bmm(out, col, first=False, last=False, k96=False):
                """Broadcast col ([48,1] or [96,1]) along the free dim of out.

                lhsT/out are bitcast to f32r (same bits as f32 here) so the
                bf16 identity rhs keys the 1.0 cyc/row transpose path.
                """
                kk = P2 if k96 else L
                nc.tensor.matmul(
                    out,
                    col.broadcast_to([kk, L]),
                    id96 if k96 else id48,
                    start=first,
                    stop=last,
                    is_transpose=True,
                    skip_group_check=True,
                )

            def smm(out, lhsT, first=False):
                """Static 96-wide refresh matmul (t-term)."""
                nc.tensor.matmul(
                    out,
                    lhsT,
                    id96,
                    start=first,
                    stop=False,
                    is_transpose=True,
                    skip_group_check=True,
                )

            # ---- emissions: xt/ef live only in this phase; their pool is
            # closed afterwards so the beta slab can reuse the space ----
            epctx = tc.tile_pool(name="ep", bufs=1)
            ep = epctx.__enter__()
            ef = ep.tile([128, KP, NTERM, 2, L], FP8)
            xt = ep.tile([128, KP, 2, BT], FP8)
            KSL = [0, 5, 10, 15, 20, 25, 30, 35, KP]
            for s in range(len(KSL) - 1):
                k0, k1 = KSL[s], KSL[s + 1]
                nc.sync.dma_start(
                    out=ef[:, k0:k1, :, :, :], in_=eT[:, k0:k1, :, :, :]
                )
                nc.sync.dma_start(out=xt[:, k0:k1, :, :], in_=x[:, k0:k1, :, :])
                for k in range(k0, k1):
                    for tm in range(NTERM):
                        # term 0 -> bank A (unscaled); terms 1-3 -> bank B
                        # (stored x256; the combine scales by 2^-8)
                        g = 0 if tm == 0 else 1
                        nc.tensor.matmul(
                            pem[:, g, :],
                            ef[:, k, tm, :, :],
                            xt[:, k, :, :],
                            start=(k == 0 and tm <= 1),
                            stop=(k == KP - 1 and tm in (0, NTERM - 1)),
                            perf_mode=mybir.MatmulPerfMode.DoubleRow,
                        )
            nc.scalar.copy(
                out=em48, in_=pem[:, 0, :].rearrange("p (b t) -> p b t", b=BLOC)
            )
            nc.vector.scalar_tensor_tensor(
                out=em48,
                in0=pem[:, 1, :].rearrange("p (b t) -> p b t", b=BLOC),
                scalar=1.0 / 256.0,
                in1=em48,
                op0=AL.mult,
                op1=AL.add,
            )
            nc.vector.tensor_sub(demf, em48[:, :, 1:], em48[:, :, 0 : T - 1])
            nc.vector.tensor_sub(ndemf, em48[:, :, 0 : T - 1], em48[:, :, 1:])
            epctx.__exit__(None, None, None)  # xt/ef space no longer needed

            # ---- scans ----
            def fwd_step(t):
                p = t + 1
                # off-path bcast term (inputs >= 2 steps old), on Pool
                if p >= 3:
                    q = p % 2
                    nc.gpsimd.tensor_tensor(
                        ndm[:, q, :],
                        demf[:, :, p - 3],
                        mslab[:, :, p - 2],
                        op=AL.subtract,
                    )
                for pair in range(2):
                    ps = pscA[:, pair, :]
                    if p == 1:
                        smm(ps[:, 0:P2], tfr, first=True)
                        bmm(ps[:, 0:P2], d0r, k96=True)
                    elif _fwd_refresh(p, pair):
                        smm(ps[:, 0:P2], tfr, first=True)
                        for sub in range(2):
                            b = 2 * pair + sub
                            blk = ps[:, L * sub : L * sub + L]
                            bmm(blk, em48[:, b, p - 2 : p - 1])
                            bmm(blk, mslab[:, b, p - 1 : p])
                    else:
                        q = p % 2
                        for sub in range(2):
                            b = 2 * pair + sub
                            blk = ps[:, L * sub : L * sub + L]
                            bmm(blk, mslab[:, b, p - 1 : p])
                for pair in range(2):
                    if p == 1 or _fwd_refresh(p, pair):
                        continue
                    ps = pscA[:, pair, :]
                    q = p % 2
                    for sub in range(2):
                        b = 2 * pair + sub
                        blk = ps[:, L * sub : L * sub + L]
                        bmm(blk, ndm[:, q, b : b + 1])
                nc.vector.tensor_reduce(
                    out=mslab[:, :, p : p + 1],
                    in_=pscA[:, :, 0:P2].rearrange("p a (s i) -> p a s i", s=2),
                    axis=AX.X,
                    op=AL.max,
                )

            def bwd_step(t):
                pos = T - 1 - t
                if pos <= T - 2:
                    # nb_pos = -demf[pos] - beta_{pos+2} (off-path, Pool)
                    q = t % 2
                    nc.gpsimd.tensor_tensor(
                        nbt[:, q, :],
                        ndemf[:, :, pos],
                        bslab[:, :, pos + 2],
                        op=AL.subtract,
                    )
                for pair in range(2):
                    pb = psbA[:, pair, :]
                    rf = _bwd_refresh(t, pair)
                    for sub in range(2):
                        b = 2 * pair + sub
                        blk = pb[:, 56 * sub : 56 * sub + L]
                        if rf:
                            nc.tensor.matmul(
                                blk,
                                tbr,
                                id48,
                                start=(sub == 0),
                                stop=False,
                                is_transpose=True,
                                skip_group_check=True,
                            )
                            bmm(blk, em48[:, b, pos : pos + 1])
                            if t > 0:
                                bmm(blk, bslab[:, b, pos + 1 : pos + 2])
                        else:
                            bmm(blk, bslab[:, b, pos + 1 : pos + 2])
                for pair in range(2):
                    if _bwd_refresh(t, pair):
                        continue
                    pb = psbA[:, pair, :]
                    q = t % 2
                    for sub in range(2):
                        b = 2 * pair + sub
                        blk = pb[:, 56 * sub : 56 * sub + L]
                        bmm(blk, nbt[:, q, b : b + 1])
                nc.vector.tensor_reduce(
                    out=bslab[:, :, pos : pos + 1],
                    in_=psbA[:, :, 0:112].rearrange("p a (s i) -> p a s i", i=56)[
                        :, :, :, 0:49
                    ],
                    axis=AX.X,
                    op=AL.max,
                )

            # middle combine-sum columns [C0, C1) become ready mid-loop
            # (mslab fills forward, bslab backward: col n ready at
            # t = max(n-1, T-1-n))
            C0, C1 = 8, 122
            for t in range(T):
                fwd_step(t)
                bwd_step(t)
                if t == T - 8:
                    nc.gpsimd.tensor_add(
                        dsl[:, :, C0:C1], mslab[:, :, C0:C1], em48[:, :, C0 - 1 : C1 - 1]
                    )
                    nc.gpsimd.tensor_add(
                        fsl[:, :, C0:C1], dsl[:, :, C0:C1], bslab[:, :, C0:C1]
                    )

            # ---- combine ----
            id128f = idb_sb

            def ptrans(out_psum, in_sb):
                nc.tensor.matmul(
                    out_psum,
                    in_sb,
                    id128f[0 : in_sb.shape[0], 0 : in_sb.shape[0]],
                    start=True,
                    stop=True,
                    is_transpose=True,
                    skip_group_check=True,
                )

            # dsl: col0 = d0, cols 1: = m + em;  fsl = dsl + beta
            # (middle columns were computed during the scan loop)
            nc.vector.tensor_add(
                dsl[:, :, 1:C0], mslab[:, :, 1:C0], em48[:, :, 0 : C0 - 1]
            )
            nc.vector.tensor_add(
                dsl[:, :, C1:], mslab[:, :, C1:], em48[:, :, C1 - 1 :]
            )
            nc.vector.tensor_copy(dsl[:, :, 0:1], d0col.broadcast_to([L, BLOC, 1]))
            nc.vector.tensor_add(
                fsl[:, :, 0:C0], dsl[:, :, 0:C0], bslab[:, :, 0:C0]
            )
            nc.vector.tensor_add(
                fsl[:, :, C1:], dsl[:, :, C1:], bslab[:, :, C1 : T + 1]
            )

            # per-b transposes to [T, 48] (fresh PSUM banks) + max/argmax
            pcb = ppcb.tile([128, 2, 512], F32, name="pcb")
            for b in range(BLOC):
                ptd = pcb[:, 0, b * L : (b + 1) * L][0:T, :]
                ptf = pcb[:, 1, b * L : (b + 1) * L][0:T, :]
                ptrans(ptd, dsl[:, b, 1:])
                ptrans(ptf, fsl[:, b, 1:])
            nc.vector.tensor_reduce(
                out=nm,
                in_=pcb[0:T, 0, 0 : BLOC * L].rearrange("p (b l) -> p b l", b=BLOC),
                axis=AX.X,
                op=AL.max,
            )
            for b in range(BLOC):
                ptf = pcb[:, 1, b * L : (b + 1) * L][0:T, :]
                nc.vector.max(fm8[:, b, :], ptf)
                nc.vector.max_index(fi8[:, b, :], fm8[:, b, :], ptf)
            nc.vector.tensor_copy(fidx, fi8[:, :, 0])

            # end_n per b from n_maxs (col 0 of delta_full maxes to 0)
            pnm = pcb[0:BLOC, 1, 192 : 192 + T]
            ptrans(pnm, nm)
            nc.vector.memset(nmb[:, 0:1], 0.0)
            nc.scalar.copy(out=nmb[:, 1:], in_=pnm)
            nc.vector.max(en8, nmb)
            nc.vector.max_index(eni8, en8, nmb)
            nc.vector.tensor_copy(endf, eni8[:, 0:1])

            # active mask act[b, n] = (n <= end_n), transposed to [T, 4]
            nc.vector.tensor_scalar(
                out=act, in0=io129[0:BLOC, :], scalar1=endf, scalar2=None,
                op0=AL.is_le,
            )
            pact = pcb[0:T, 0, 192 : 192 + BLOC]
            ptrans(pact, act[:, 1:])
            nc.scalar.copy(out=actT, in_=pact)

            # y one-hots
            for b in range(BLOC):
                nc.vector.tensor_scalar(
                    out=ybig[:, b, :],
                    in0=io48[0:T, :],
                    scalar1=fidx[:, b : b + 1],
                    scalar2=actT[:, b : b + 1],
                    op0=AL.is_equal,
                    op1=AL.mult,
                )
            nc.sync.dma_start(out=y[:, :, :], in_=ybig)

    nc.finalize()
    _legalize_sync_waits(nc)
    return nc


def _legalize_sync_waits(nc):
    """This container's walrus accepts at most ONE sync wait per instruction.

    Split excess waits onto Drain instructions inserted just before the
    offending instruction (same engine, so the waits still complete before it
    issues; an idle-pipe Drain costs ~12ns).  Applied to the serialized BIR
    only -- CoreSim consumes the in-memory module and is unaffected.
    """
    import json as _json

    m = _json.loads(nc.to_json_bytes())
    for f in m["functions"]:
        for blk in f["blocks"]:
            out = []
            for ins in blk["instructions"]:
                si = ins.get("sync_info") or {}
                w = si.get("on_wait") or []
                if len(w) > 1:
                    for j, wx in enumerate(w[:-1]):
                        out.append(
                            {
                                "debug": ins.get("debug", 0),
                                "engine": ins["engine"],
                                "ins": [],
                                "outs": [],
                                "name": f"{ins['name']}-w{j}",
                                "opcode": "Drain",
                                "sync_info": {"on_update": [], "on_wait": [wx]},
                            }
                        )
                    si["on_wait"] = [w[-1]]
                out.append(ins)
            blk["instructions"] = out
    blob = _json.dumps(m).encode()
    nc.to_json_bytes = lambda: blob


def make_consts():
    f32 = np.float32
    c = np.zeros((128, CW), f32)
    c[0:L, 97] = NEG
    c[0, 97] = 0.0
    d0 = c[0:L, 97].copy()
    c[0:P2, 96] = np.concatenate([d0, d0])
    c[0:L, 98:227] = np.arange(T + 1, dtype=f32)[None, :]
    c[:, 227:275] = np.arange(L, dtype=f32)[None, :]
    return c


def make_in_maps(X, t_feats, e_feats):
    f32 = np.float32
    t_feats = np.asarray(t_feats, dtype=f32)
    e_feats = np.asarray(e_feats, dtype=f32)
    c = make_consts()
    c[0:P2, 0:L] = np.vstack([t_feats, t_feats])
    c[0:L, L : 2 * L] = t_feats.T

    idb = np.eye(128, dtype=f32)

    # e blob [v%128, kp, term, j, L] fp8: 4-term exact-to-~2^-16 split of e^T
    fp8 = mybir.dt.np(FP8)
    eTf = np.zeros((NK * 128, L), f32)
    eTf[:V] = np.ascontiguousarray(e_feats.T)
    terms = []
    t0 = eTf.astype(fp8)
    terms.append(t0)
    rs = (eTf - t0.astype(f32)) * 256.0
    for _ in range(NTERM - 1):
        t = rs.astype(fp8)
        terms.append(t)
        rs = rs - t.astype(f32)
    efm = np.ascontiguousarray(
        np.stack(terms, axis=1)              # [NK*128, NTERM, L]
        .reshape(KP, 2, 128, NTERM, L)       # [kp, j, p, term, L]
        .transpose(2, 0, 3, 1, 4)            # [p, kp, term, j, L]
    )

    # x blob per core [v%128, kp, j, (b t)] in fp8 (one-hot: exact)
    X = np.asarray(X)
    in_maps = []
    for ci in range(NCORES):
        Xc = np.zeros((BLOC, T, NK * 128), f32)
        Xc[:, :, :V] = X[ci * BLOC : (ci + 1) * BLOC]
        # [b, t, kp, j, p] -> [p, kp, j, b, t]
        xb = np.ascontiguousarray(
            Xc.reshape(BLOC, T, KP, 2, 128)
            .transpose(4, 2, 3, 0, 1)
            .reshape(128, KP, 2, BT)
        ).astype(fp8)
        in_maps.append({"x": xb, "eT": efm, "consts": c, "idb": idb})
    return in_maps


_NC = None


def _get_nc():
    global _NC
    if _NC is None:
        _NC = build_nc()
    return _NC


def kernel(X, t_feats, e_feats):
    in_maps = make_in_maps(X, t_feats, e_feats)
    nc = _get_nc()
    res = run_bass_kernel_spmd(nc, in_maps, list(range(NCORES)))
    out = np.concatenate(
        [res.results[ci]["y"].transpose(1, 0, 2) for ci in range(NCORES)], axis=0
    )
    return np.ascontiguousarray(out, dtype=np.float32)


# revision 34
# speedup vs baseline: 1.0030x; 1.0030x over previous
"""CRF Viterbi decode kernel for Trainium2 (Bass), data-parallel over batch.

Problem shapes (hardcoded): X [32,128,10000] f32 one-hot, t_feats [48,48],
e_feats [48,10000].  Output Y_hat [32,128,48] f32 one-hot.

Sharding: batch 32 -> 8 cores x 4.  t_feats / e_feats replicated.

Per-core pipeline (4 batch elems, labels on 48 partitions throughout):
  1. emissions em[l, (b,t)] = e_feats @ X^T: the host pre-transposes the
     one-hot X into an fp8 [v%128, kpair, j, (b t)] blob (0/1 exact in
     fp8) and splits e_feats^T into 4 exact fp8 terms (term 0 unscaled
     into PSUM bank A; terms 1-3 in 2^8-scaled space into bank B;
     combined as A + 2^-8 B).  Emissions are then 160 DoubleRow fp8
     matmuls (2 v-chunks per pass, 0.5 cyc/row) chasing the sliced X
     DMA -- no on-device transposes or staging.
  2. forward Viterbi: per pair of batch elems a PSUM-resident score
     tile psc[j, (sub i)] = t[i,j] + delta[i], updated per step with
     two 48-col f32 broadcast matmuls per elem: on-path bcast(m_{p-1})
     and off-path bcast(demf[p-3] - m_{p-2}) (2-step slack, built on
     the idle Pool engine).  One merged DVE tensor_reduce over both
     pairs' banks produces m_p for all 4 elems.  Periodic staggered
     full refreshes bound fp drift (exact f32 elsewhere).
  3. backward "end-anytime" scan, same structure with t^T, 56-stride
     segments whose permanent zero columns implement beta = max(0, .).
     Runs anti-phased with the forward scan: per ~1us step the two
     chains interleave PE broadcast clusters with DVE reduces.
  4. combine: y_n = onehot(argmax_j delta_n(j)+beta_n(j)) * [n <=
     end_n] via per-elem PE transposes + DVE max/max_index (first-index
     tie semantics match the reference argmax); the middle score
     columns are pre-summed during the scan loop.  Output is written
     label-minor [T, B, L] and unscrambled on the host.

Numerics notes for this stack: float32r matmuls round their inputs on
the NEFF/ucode path (CoreSim does not model it), so every scan matmul
stays plain f32; fp8/bf16 matmul inputs are exact by construction.
GPSIMD cannot touch PSUM, scalar_tensor_tensor does not exist on Pool,
and partition_all_reduce does not compile -- hence the DVE-only
reduces and Pool doing only small SBUF tensor_tensor work.
"""

import os
import sys

import numpy as np

for _p in ("/opt/trn_rl_repo",):
    if _p not in sys.path and os.path.isdir(_p):
        sys.path.insert(0, _p)

import concourse.bass as bass
import concourse.tile as tile
from concourse import mybir
from concourse.bass_utils import run_bass_kernel_spmd

F32 = mybir.dt.float32
F32R = mybir.dt.float32r
BF16 = mybir.dt.bfloat16
FP8 = mybir.dt.float8e4
AL = mybir.AluOpType
AX = mybir.AxisListType

B, T, V, L = 32, 128, 10000, 48
NCORES = 8
BLOC = B // NCORES          # 4 batch elems per core
NK = 80                     # V chunks of 128 (tail zero-padded; even for DoubleRow)
KP = NK // 2                # DoubleRow processes 2 chunks per matmul
NTERM = 4                   # fp8 e-term count (exact split of e_feats)
BT = BLOC * T               # 512 moving columns, b-major
NEG = -1.0e30
P2 = 2 * L                  # 96

# consts layout [128, CW] f32:
#  cols 0:48     tfS   [96,48]  vstack(t, t)        (fwd refresh lhsT)
#  cols 48:96    tbr   [48,48]  t.T                 (bwd refresh lhsT)
#  col  96       d0stack [96,1]
#  col  97       d0col   [48,1]
#  cols 98:227   iota129 [48,129]
#  cols 227:275  iota48  [128,48]
CW = 275

# fwd refresh schedule: always at p==2 (clears the +-1e30 d0 arithmetic),
# then every 16 steps staggered per pair.  bwd staggered likewise on t.
RF = 16


def _fwd_refresh(p, pair):
    if p == 2:
        return True
    return p > 2 and (p - 2 - 8 * pair) % RF == 0


def _bwd_refresh(t, pair):
    if t == 0:
        return True
    return (t - 6 - 8 * pair) % RF == 0


def build_nc():
    nc = bass.Bass()

    x = nc.declare_dram_parameter("x", [128, KP, 2, BT], FP8, isOutput=False)
    eT = nc.declare_dram_parameter("eT", [128, KP, NTERM, 2, L], FP8, isOutput=False)
    consts = nc.declare_dram_parameter("consts", [128, CW], F32, isOutput=False)
    idb = nc.declare_dram_parameter("idb", [128, 128], F32, isOutput=False)
    y = nc.declare_dram_parameter("y", [T, BLOC, L], F32, isOutput=True)

    with tile.TileContext(nc) as tc:
        from contextlib import ExitStack

        with ExitStack() as ctx:
            cons = ctx.enter_context(tc.tile_pool(name="cons", bufs=1))
            pers = ctx.enter_context(tc.tile_pool(name="pers", bufs=1))
            ppem = ctx.enter_context(tc.tile_pool(name="ppem", bufs=1, space="PSUM"))
            ppfw = ctx.enter_context(tc.tile_pool(name="ppfw", bufs=1, space="PSUM"))
            ppbw = ctx.enter_context(tc.tile_pool(name="ppbw", bufs=1, space="PSUM"))
            ppcb = ctx.enter_context(tc.tile_pool(name="ppcb", bufs=1, space="PSUM"))

            # ---- constants ----
            cons_sb = cons.tile([128, CW], F32)
            nc.sync.dma_start(out=cons_sb, in_=consts[:, :])
            d0col = cons_sb[0:L, 97:98]
            io129 = cons_sb[0:L, 98:227]
            io48 = cons_sb[:, 227:275]

            tfr = cons_sb[0:P2, 0:L]
            tbr = cons_sb[0:L, L : 2 * L]
            d0r = cons_sb[0:P2, 96:97]

            idb_sb = cons.tile([128, 128], F32)
            nc.sync.dma_start(out=idb_sb, in_=idb[:, :])
            id96 = idb_sb[0:P2, 0:P2]
            id48 = idb_sb[0:L, 0:L]

            # ---- persistent state ----
            em48 = pers.tile([L, BLOC, T], F32, name="em48")
            demf = pers.tile([L, BLOC, T - 1], F32, name="demf")
            ndemf = pers.tile([L, BLOC, T - 1], F32, name="ndemf")
            mslab = pers.tile([L, BLOC, T + 1], F32, name="mslab")
            bslab = pers.tile([L, BLOC, T + 2], F32, name="bslab")
            ndm = pers.tile([L, 2, BLOC], F32, name="ndm")
            nbt = pers.tile([L, 2, BLOC], F32, name="nbt")
            dsl = pers.tile([L, BLOC, T + 1], F32, name="dsl")
            fsl = pers.tile([L, BLOC, T + 1], F32, name="fsl")
            fi8 = pers.tile([T, BLOC, 8], mybir.dt.uint32, name="fi8")
            fm8 = pers.tile([T, BLOC, 8], F32, name="fm8")
            fidx = pers.tile([T, BLOC], F32, name="fidx")
            nm = pers.tile([T, BLOC], F32, name="nm")
            nmb = pers.tile([BLOC, T + 1], F32, name="nmb")
            en8 = pers.tile([BLOC, 8], F32, name="en8")
            eni8 = pers.tile([BLOC, 8], mybir.dt.uint32, name="eni8")
            endf = pers.tile([BLOC, 1], F32, name="endf")
            act = pers.tile([BLOC, T + 1], F32, name="act")
            actT = pers.tile([T, BLOC], F32, name="actT")
            ybig = pers.tile([T, BLOC, L], F32, name="ybig")

            pscA = ppfw.tile([L, 2, 512], F32, name="pscA")
            psbA = ppbw.tile([L, 2, 512], F32, name="psbA")
            pem = ppem.tile([L, 2, 512], F32, name="pem")
            # bwd uses 49-wide segments; cols 48/97 stay 0 forever (the
            # max-with-zero floor).  beta_T = 0 (bslab col T).
            nc.vector.memset(psbA[:, :, 48:105:56], 0.0)
            nc.vector.memset(bslab[:, :, T : T + 1], 0.0)
            nc.vector.memset(bslab[:, :, 0:1], 0.0)

            def bmm(out, col, first=False, last=False, k96=False):
                """Broadcast col ([48,1] or [96,1]) along the free dim of out.

                lhsT/out are bitcast to f32r (same bits as f32 here) so the
                bf16 identity rhs keys the 1.0 cyc/row transpose path.
                """
                kk = P2 if k96 else L
                nc.tensor.matmul(
                    out,
                    col.broadcast_to([kk, L]),
                    id96 if k96 else id48,
                    start=first,
                    stop=last,
                    is_transpose=True,
                    skip_group_check=True,
                )

            def smm(out, lhsT, first=False):
                """Static 96-wide refresh matmul (t-term)."""
                nc.tensor.matmul(
                    out,
                    lhsT,
                    id96,
                    start=first,
                    stop=False,
                    is_transpose=True,
                    skip_group_check=True,
                )

            # ---- emissions: xt/ef live only in this phase; their pool is
            # closed afterwards so the beta slab can reuse the space ----
            epctx = tc.tile_pool(name="ep", bufs=1)
            ep = epctx.__enter__()
            ef = ep.tile([128, KP, NTERM, 2, L], FP8)
            xt = ep.tile([128, KP, 2, BT], FP8)
            KSL = [0, 5, 10, 15, 20, 25, 30, 35, KP]
            for s in range(len(KSL) - 1):
                k0, k1 = KSL[s], KSL[s + 1]
                nc.sync.dma_start(
                    out=ef[:, k0:k1, :, :, :], in_=eT[:, k0:k1, :, :, :]
                )
                nc.sync.dma_start(out=xt[:, k0:k1, :, :], in_=x[:, k0:k1, :, :])
                for k in range(k0, k1):
                    for tm in range(NTERM):
                        # term 0 -> bank A (unscaled); terms 1-3 -> bank B
                        # (stored x256; the combine scales by 2^-8)
                        g = 0 if tm == 0 else 1
                        nc.tensor.matmul(
                            pem[:, g, :],
                            ef[:, k, tm, :, :],
                            xt[:, k, :, :],
                            start=(k == 0 and tm <= 1),
                            stop=(k == KP - 1 and tm in (0, NTERM - 1)),
                            perf_mode=mybir.MatmulPerfMode.DoubleRow,
                        )
            nc.scalar.copy(
                out=em48, in_=pem[:, 0, :].rearrange("p (b t) -> p b t", b=BLOC)
            )
            nc.vector.scalar_tensor_tensor(
                out=em48,
                in0=pem[:, 1, :].rearrange("p (b t) -> p b t", b=BLOC),
                scalar=1.0 / 256.0,
                in1=em48,
                op0=AL.mult,
                op1=AL.add,
            )
            nc.vector.tensor_sub(demf, em48[:, :, 1:], em48[:, :, 0 : T - 1])
            nc.vector.tensor_sub(ndemf, em48[:, :, 0 : T - 1], em48[:, :, 1:])
            epctx.__exit__(None, None, None)  # xt/ef space no longer needed

            # ---- scans ----
            def fwd_step(t):
                p = t + 1
                # off-path bcast term (inputs >= 2 steps old), on Pool
                if p >= 3:
                    q = p % 2
                    nc.gpsimd.tensor_tensor(
                        ndm[:, q, :],
                        demf[:, :, p - 3],
                        mslab[:, :, p - 2],
                        op=AL.subtract,
                    )
                for pair in range(2):
                    ps = pscA[:, pair, :]
                    if p == 1:
                        smm(ps[:, 0:P2], tfr, first=True)
                        bmm(ps[:, 0:P2], d0r, k96=True)
                    elif _fwd_refresh(p, pair):
                        smm(ps[:, 0:P2], tfr, first=True)
                        for sub in range(2):
                            b = 2 * pair + sub
                            blk = ps[:, L * sub : L * sub + L]
                            bmm(blk, em48[:, b, p - 2 : p - 1])
                            bmm(blk, mslab[:, b, p - 1 : p])
                    else:
                        q = p % 2
                        for sub in range(2):
                            b = 2 * pair + sub
                            blk = ps[:, L * sub : L * sub + L]
                            bmm(blk, mslab[:, b, p - 1 : p])
                for pair in range(2):
                    if p == 1 or _fwd_refresh(p, pair):
                        continue
                    ps = pscA[:, pair, :]
                    q = p % 2
                    for sub in range(2):
                        b = 2 * pair + sub
                        blk = ps[:, L * sub : L * sub + L]
                        bmm(blk, ndm[:, q, b : b + 1])
                nc.vector.tensor_reduce(
                    out=mslab[:, :, p : p + 1],
                    in_=pscA[:, :, 0:P2].rearrange("p a (s i) -> p a s i", s=2),
                    axis=AX.X,
                    op=AL.max,
                )

            def bwd_step(t):
                pos = T - 1 - t
                if pos <= T - 2:
                    # nb_pos = -demf[pos] - beta_{pos+2} (off-path, Pool)
                    q = t % 2
                    nc.gpsimd.tensor_tensor(
                        nbt[:, q, :],
                        ndemf[:, :, pos],
                        bslab[:, :, pos + 2],
                        op=AL.subtract,
                    )
                for pair in range(2):
                    pb = psbA[:, pair, :]
                    rf = _bwd_refresh(t, pair)
                    for sub in range(2):
                        b = 2 * pair + sub
                        blk = pb[:, 56 * sub : 56 * sub + L]
                        if rf:
                            nc.tensor.matmul(
                                blk,
                                tbr,
                                id48,
                                start=(sub == 0),
                                stop=False,
                                is_transpose=True,
                                skip_group_check=True,
                            )
                            bmm(blk, em48[:, b, pos : pos + 1])
                            if t > 0:
                                bmm(blk, bslab[:, b, pos + 1 : pos + 2])
                        else:
                            bmm(blk, bslab[:, b, pos + 1 : pos + 2])
                for pair in range(2):
                    if _bwd_refresh(t, pair):
                        continue
                    pb = psbA[:, pair, :]
                    q = t % 2
                    for sub in range(2):
                        b = 2 * pair + sub
                        blk = pb[:, 56 * sub : 56 * sub + L]
                        bmm(blk, nbt[:, q, b : b + 1])
                nc.vector.tensor_reduce(
                    out=bslab[:, :, pos : pos + 1],
                    in_=psbA[:, :, 0:112].rearrange("p a (s i) -> p a s i", i=56)[
                        :, :, :, 0:49
                    ],
                    axis=AX.X,
                    op=AL.max,
                )

            # middle combine-sum columns [C0, C1) become ready mid-loop
            # (mslab fills forward, bslab backward: col n ready at
            # t = max(n-1, T-1-n))
            C0, C1 = 8, 122
            for t in range(T):
                fwd_step(t)
                if t < T - 1:  # beta_0 (t=127) is never consumed
                    bwd_step(t)
                if t == T - 8:
                    nc.gpsimd.tensor_add(
                        dsl[:, :, C0:C1], mslab[:, :, C0:C1], em48[:, :, C0 - 1 : C1 - 1]
                    )
                    nc.gpsimd.tensor_add(
                        fsl[:, :, C0:C1], dsl[:, :, C0:C1], bslab[:, :, C0:C1]
                    )

            # ---- combine ----
            id128f = idb_sb

            def ptrans(out_psum, in_sb):
                nc.tensor.matmul(
                    out_psum,
                    in_sb,
                    id128f[0 : in_sb.shape[0], 0 : in_sb.shape[0]],
                    start=True,
                    stop=True,
                    is_transpose=True,
                    skip_group_check=True,
                )

            # dsl: col0 = d0, cols 1: = m + em;  fsl = dsl + beta
            # (middle columns were computed during the scan loop)
            nc.vector.tensor_add(
                dsl[:, :, 1:C0], mslab[:, :, 1:C0], em48[:, :, 0 : C0 - 1]
            )
            nc.vector.tensor_add(
                dsl[:, :, C1:], mslab[:, :, C1:], em48[:, :, C1 - 1 :]
            )
            nc.vector.tensor_copy(dsl[:, :, 0:1], d0col.broadcast_to([L, BLOC, 1]))
            nc.vector.tensor_add(
                fsl[:, :, 0:C0], dsl[:, :, 0:C0], bslab[:, :, 0:C0]
            )
            nc.vector.tensor_add(
                fsl[:, :, C1:], dsl[:, :, C1:], bslab[:, :, C1 : T + 1]
            )

            # per-b transposes to [T, 48] (fresh PSUM banks) + max/argmax
            pcb = ppcb.tile([128, 2, 512], F32, name="pcb")
            for b in range(BLOC):
                ptd = pcb[:, 0, b * L : (b + 1) * L][0:T, :]
                ptf = pcb[:, 1, b * L : (b + 1) * L][0:T, :]
                ptrans(ptd, dsl[:, b, 1:])
                ptrans(ptf, fsl[:, b, 1:])
            nc.vector.tensor_reduce(
                out=nm,
                in_=pcb[0:T, 0, 0 : BLOC * L].rearrange("p (b l) -> p b l", b=BLOC),
                axis=AX.X,
                op=AL.max,
            )
            for b in range(BLOC):
                ptf = pcb[:, 1, b * L : (b + 1) * L][0:T, :]
                nc.vector.max(fm8[:, b, :], ptf)
                nc.vector.max_index(fi8[:, b, :], fm8[:, b, :], ptf)
            nc.vector.tensor_copy(fidx, fi8[:, :, 0])

            # end_n per b from n_maxs (col 0 of delta_full maxes to 0)
            pnm = pcb[0:BLOC, 1, 192 : 192 + T]
            ptrans(pnm, nm)
            nc.vector.memset(nmb[:, 0:1], 0.0)
            nc.scalar.copy(out=nmb[:, 1:], in_=pnm)
            nc.vector.max(en8, nmb)
            nc.vector.max_index(eni8, en8, nmb)
            nc.vector.tensor_copy(endf, eni8[:, 0:1])

            # active mask act[b, n] = (n <= end_n), transposed to [T, 4]
            nc.vector.tensor_scalar(
                out=act, in0=io129[0:BLOC, :], scalar1=endf, scalar2=None,
                op0=AL.is_le,
            )
            pact = pcb[0:T, 0, 192 : 192 + BLOC]
            ptrans(pact, act[:, 1:])
            nc.scalar.copy(out=actT, in_=pact)

            # y one-hots
            for b in range(BLOC):
                nc.vector.tensor_scalar(
                    out=ybig[:, b, :],
                    in0=io48[0:T, :],
                    scalar1=fidx[:, b : b + 1],
                    scalar2=actT[:, b : b + 1],
                    op0=AL.is_equal,
                    op1=AL.mult,
                )
            nc.sync.dma_start(out=y[:, :, :], in_=ybig)

    nc.finalize()
    _legalize_sync_waits(nc)
    return nc


def _legalize_sync_waits(nc):
    """This container's walrus accepts at most ONE sync wait per instruction.

    Split excess waits onto Drain instructions inserted just before the
    offending instruction (same engine, so the waits still complete before it
    issues; an idle-pipe Drain costs ~12ns).  Applied to the serialized BIR
    only -- CoreSim consumes the in-memory module and is unaffected.
    """
    import json as _json

    m = _json.loads(nc.to_json_bytes())
    for f in m["functions"]:
        for blk in f["blocks"]:
            out = []
            for ins in blk["instructions"]:
                si = ins.get("sync_info") or {}
                w = si.get("on_wait") or []
                if len(w) > 1:
                    for j, wx in enumerate(w[:-1]):
                        out.append(
                            {
                                "debug": ins.get("debug", 0),
                                "engine": ins["engine"],
                                "ins": [],
                                "outs": [],
                                "name": f"{ins['name']}-w{j}",
                                "opcode": "Drain",
                                "sync_info": {"on_update": [], "on_wait": [wx]},
                            }
                        )
                    si["on_wait"] = [w[-1]]
                out.append(ins)
            blk["instructions"] = out
    blob = _json.dumps(m).encode()
    nc.to_json_bytes = lambda: blob


def make_consts():
    f32 = np.float32
    c = np.zeros((128, CW), f32)
    c[0:L, 97] = NEG
    c[0, 97] = 0.0
    d0 = c[0:L, 97].copy()
    c[0:P2, 96] = np.concatenate([d0, d0])
    c[0:L, 98:227] = np.arange(T + 1, dtype=f32)[None, :]
    c[:, 227:275] = np.arange(L, dtype=f32)[None, :]
    return c


def make_in_maps(X, t_feats, e_feats):
    f32 = np.float32
    t_feats = np.asarray(t_feats, dtype=f32)
    e_feats = np.asarray(e_feats, dtype=f32)
    c = make_consts()
    c[0:P2, 0:L] = np.vstack([t_feats, t_feats])
    c[0:L, L : 2 * L] = t_feats.T

    idb = np.eye(128, dtype=f32)

    # e blob [v%128, kp, term, j, L] fp8: 4-term exact-to-~2^-16 split of e^T
    fp8 = mybir.dt.np(FP8)
    eTf = np.zeros((NK * 128, L), f32)
    eTf[:V] = np.ascontiguousarray(e_feats.T)
    terms = []
    t0 = eTf.astype(fp8)
    terms.append(t0)
    rs = (eTf - t0.astype(f32)) * 256.0
    for _ in range(NTERM - 1):
        t = rs.astype(fp8)
        terms.append(t)
        rs = rs - t.astype(f32)
    efm = np.ascontiguousarray(
        np.stack(terms, axis=1)              # [NK*128, NTERM, L]
        .reshape(KP, 2, 128, NTERM, L)       # [kp, j, p, term, L]
        .transpose(2, 0, 3, 1, 4)            # [p, kp, term, j, L]
    )

    # x blob per core [v%128, kp, j, (b t)] in fp8 (one-hot: exact)
    X = np.asarray(X)
    in_maps = []
    for ci in range(NCORES):
        Xc = np.zeros((BLOC, T, NK * 128), f32)
        Xc[:, :, :V] = X[ci * BLOC : (ci + 1) * BLOC]
        # [b, t, kp, j, p] -> [p, kp, j, b, t]
        xb = np.ascontiguousarray(
            Xc.reshape(BLOC, T, KP, 2, 128)
            .transpose(4, 2, 3, 0, 1)
            .reshape(128, KP, 2, BT)
        ).astype(fp8)
        in_maps.append({"x": xb, "eT": efm, "consts": c, "idb": idb})
    return in_maps


_NC = None


def _get_nc():
    global _NC
    if _NC is None:
        _NC = build_nc()
    return _NC


def kernel(X, t_feats, e_feats):
    in_maps = make_in_maps(X, t_feats, e_feats)
    nc = _get_nc()
    res = run_bass_kernel_spmd(nc, in_maps, list(range(NCORES)))
    out = np.concatenate(
        [res.results[ci]["y"].transpose(1, 0, 2) for ci in range(NCORES)], axis=0
    )
    return np.ascontiguousarray(out, dtype=np.float32)


# revision 35
# speedup vs baseline: 1.0059x; 1.0030x over previous
"""CRF Viterbi decode kernel for Trainium2 (Bass), data-parallel over batch.

Problem shapes (hardcoded): X [32,128,10000] f32 one-hot, t_feats [48,48],
e_feats [48,10000].  Output Y_hat [32,128,48] f32 one-hot.

Sharding: batch 32 -> 8 cores x 4.  t_feats / e_feats replicated.

Per-core pipeline (4 batch elems, labels on 48 partitions throughout):
  1. emissions em[l, (b,t)] = e_feats @ X^T: the host pre-transposes the
     one-hot X into an fp8 [v%128, kpair, j, (b t)] blob (0/1 exact in
     fp8) and splits e_feats^T into 4 exact fp8 terms (term 0 unscaled
     into PSUM bank A; terms 1-3 in 2^8-scaled space into bank B;
     combined as A + 2^-8 B).  Emissions are then 160 DoubleRow fp8
     matmuls (2 v-chunks per pass, 0.5 cyc/row) chasing the sliced X
     DMA -- no on-device transposes or staging.
  2. forward Viterbi: per pair of batch elems a PSUM-resident score
     tile psc[j, (sub i)] = t[i,j] + delta[i], updated per step with
     two 48-col f32 broadcast matmuls per elem: on-path bcast(m_{p-1})
     and off-path bcast(demf[p-3] - m_{p-2}) (2-step slack, built on
     the idle Pool engine).  One merged DVE tensor_reduce over both
     pairs' banks produces m_p for all 4 elems.  Periodic staggered
     full refreshes bound fp drift (exact f32 elsewhere).
  3. backward "end-anytime" scan, same structure with t^T, 56-stride
     segments whose permanent zero columns implement beta = max(0, .).
     Runs anti-phased with the forward scan: per ~1us step the two
     chains interleave PE broadcast clusters with DVE reduces.
  4. combine: y_n = onehot(argmax_j delta_n(j)+beta_n(j)) * [n <=
     end_n] via per-elem PE transposes + DVE max/max_index (first-index
     tie semantics match the reference argmax); the middle score
     columns are pre-summed during the scan loop.  Output is written
     label-minor [T, B, L] and unscrambled on the host.

Numerics notes for this stack: float32r matmuls round their inputs on
the NEFF/ucode path (CoreSim does not model it), so every scan matmul
stays plain f32; fp8/bf16 matmul inputs are exact by construction.
GPSIMD cannot touch PSUM, scalar_tensor_tensor does not exist on Pool,
and partition_all_reduce does not compile -- hence the DVE-only
reduces and Pool doing only small SBUF tensor_tensor work.
"""

import os
import sys

import numpy as np

for _p in ("/opt/trn_rl_repo",):
    if _p not in sys.path and os.path.isdir(_p):
        sys.path.insert(0, _p)

import concourse.bass as bass
import concourse.tile as tile
from concourse import mybir
from concourse.bass_utils import run_bass_kernel_spmd

F32 = mybir.dt.float32
F32R = mybir.dt.float32r
BF16 = mybir.dt.bfloat16
FP8 = mybir.dt.float8e4
AL = mybir.AluOpType
AX = mybir.AxisListType

B, T, V, L = 32, 128, 10000, 48
NCORES = 8
BLOC = B // NCORES          # 4 batch elems per core
NK = 80                     # V chunks of 128 (tail zero-padded; even for DoubleRow)
KP = NK // 2                # DoubleRow processes 2 chunks per matmul
NTERM = 4                   # fp8 e-term count (exact split of e_feats)
BT = BLOC * T               # 512 moving columns, b-major
NEG = -1.0e30
P2 = 2 * L                  # 96

# consts layout [128, CW] f32:
#  cols 0:48     tfS   [96,48]  vstack(t, t)        (fwd refresh lhsT)
#  cols 48:96    tbr   [48,48]  t.T                 (bwd refresh lhsT)
#  col  96       d0stack [96,1]
#  col  97       d0col   [48,1]
#  cols 98:227   iota129 [48,129]
#  cols 227:275  iota48  [128,48]
CW = 275

# fwd refresh schedule: always at p==2 (clears the +-1e30 d0 arithmetic),
# then every 16 steps staggered per pair.  bwd staggered likewise on t.
RF = 16


def _fwd_refresh(p, pair):
    if p == 2:
        return True
    return p > 2 and (p - 2 - 8 * pair) % RF == 0


def _bwd_refresh(t, pair):
    if t == 0:
        return True
    return (t - 6 - 8 * pair) % RF == 0


def build_nc():
    nc = bass.Bass()

    x = nc.declare_dram_parameter("x", [128, KP, 2, BT], FP8, isOutput=False)
    eT = nc.declare_dram_parameter("eT", [128, KP, NTERM, 2, L], FP8, isOutput=False)
    consts = nc.declare_dram_parameter("consts", [128, CW], F32, isOutput=False)
    idb = nc.declare_dram_parameter("idb", [128, 128], F32, isOutput=False)
    y = nc.declare_dram_parameter("y", [T, BLOC, L], F32, isOutput=True)

    with tile.TileContext(nc) as tc:
        from contextlib import ExitStack

        with ExitStack() as ctx:
            cons = ctx.enter_context(tc.tile_pool(name="cons", bufs=1))
            pers = ctx.enter_context(tc.tile_pool(name="pers", bufs=1))
            ppem = ctx.enter_context(tc.tile_pool(name="ppem", bufs=1, space="PSUM"))
            ppfw = ctx.enter_context(tc.tile_pool(name="ppfw", bufs=1, space="PSUM"))
            ppbw = ctx.enter_context(tc.tile_pool(name="ppbw", bufs=1, space="PSUM"))
            ppcb = ctx.enter_context(tc.tile_pool(name="ppcb", bufs=1, space="PSUM"))

            # ---- constants ----
            cons_sb = cons.tile([128, CW], F32)
            nc.sync.dma_start(out=cons_sb, in_=consts[:, :])
            d0col = cons_sb[0:L, 97:98]
            io129 = cons_sb[0:L, 98:227]
            io48 = cons_sb[:, 227:275]

            tfr = cons_sb[0:P2, 0:L]
            tbr = cons_sb[0:L, L : 2 * L]
            d0r = cons_sb[0:P2, 96:97]

            idb_sb = cons.tile([128, 128], F32)
            nc.sync.dma_start(out=idb_sb, in_=idb[:, :])
            id96 = idb_sb[0:P2, 0:P2]
            id48 = idb_sb[0:L, 0:L]

            # ---- persistent state ----
            em48 = pers.tile([L, BLOC, T], F32, name="em48")
            demf = pers.tile([L, BLOC, T - 1], F32, name="demf")
            ndemf = pers.tile([L, BLOC, T - 1], F32, name="ndemf")
            mslab = pers.tile([L, BLOC, T + 1], F32, name="mslab")
            bslab = pers.tile([L, BLOC, T + 2], F32, name="bslab")
            ndm = pers.tile([L, 2, BLOC], F32, name="ndm")
            nbt = pers.tile([L, 2, BLOC], F32, name="nbt")
            dsl = pers.tile([L, BLOC, T + 1], F32, name="dsl")
            fsl = pers.tile([L, BLOC, T + 1], F32, name="fsl")
            fi8 = pers.tile([T, BLOC, 8], mybir.dt.uint32, name="fi8")
            fm8 = pers.tile([T, BLOC, 8], F32, name="fm8")
            fidx = pers.tile([T, BLOC], F32, name="fidx")
            nm = pers.tile([T, BLOC], F32, name="nm")
            nmb = pers.tile([BLOC, T + 1], F32, name="nmb")
            en8 = pers.tile([BLOC, 8], F32, name="en8")
            eni8 = pers.tile([BLOC, 8], mybir.dt.uint32, name="eni8")
            endf = pers.tile([BLOC, 1], F32, name="endf")
            act = pers.tile([BLOC, T + 1], F32, name="act")
            actT = pers.tile([T, BLOC], F32, name="actT")
            ybig = pers.tile([T, BLOC, L], F32, name="ybig")

            pscA = ppfw.tile([L, 2, 512], F32, name="pscA")
            psbA = ppbw.tile([L, 2, 512], F32, name="psbA")
            pem = ppem.tile([L, 2, 512], F32, name="pem")
            # bwd uses 49-wide segments; cols 48/97 stay 0 forever (the
            # max-with-zero floor).  beta_T = 0 (bslab col T).
            nc.vector.memset(psbA[:, :, 48:105:56], 0.0)
            nc.vector.memset(bslab[:, :, T : T + 1], 0.0)
            nc.vector.memset(bslab[:, :, 0:1], 0.0)

            def bmm(out, col, first=False, last=False, k96=False):
                """Broadcast col ([48,1] or [96,1]) along the free dim of out.

                lhsT/out are bitcast to f32r (same bits as f32 here) so the
                bf16 identity rhs keys the 1.0 cyc/row transpose path.
                """
                kk = P2 if k96 else L
                nc.tensor.matmul(
                    out,
                    col.broadcast_to([kk, L]),
                    id96 if k96 else id48,
                    start=first,
                    stop=last,
                    is_transpose=True,
                    skip_group_check=True,
                )

            def smm(out, lhsT, first=False):
                """Static 96-wide refresh matmul (t-term)."""
                nc.tensor.matmul(
                    out,
                    lhsT,
                    id96,
                    start=first,
                    stop=False,
                    is_transpose=True,
                    skip_group_check=True,
                )

            # ---- emissions: xt/ef live only in this phase; their pool is
            # closed afterwards so the beta slab can reuse the space ----
            epctx = tc.tile_pool(name="ep", bufs=1)
            ep = epctx.__enter__()
            ef = ep.tile([128, KP, NTERM, 2, L], FP8)
            xt = ep.tile([128, KP, 2, BT], FP8)
            KSL = [0, 5, 10, 15, 20, 25, 30, 35, KP]
            for s in range(len(KSL) - 1):
                k0, k1 = KSL[s], KSL[s + 1]
                nc.sync.dma_start(
                    out=ef[:, k0:k1, :, :, :], in_=eT[:, k0:k1, :, :, :]
                )
                nc.sync.dma_start(out=xt[:, k0:k1, :, :], in_=x[:, k0:k1, :, :])
                for k in range(k0, k1):
                    for tm in range(NTERM):
                        # term 0 -> bank A (unscaled); terms 1-3 -> bank B
                        # (stored x256; the combine scales by 2^-8)
                        g = 0 if tm == 0 else 1
                        nc.tensor.matmul(
                            pem[:, g, :],
                            ef[:, k, tm, :, :],
                            xt[:, k, :, :],
                            start=(k == 0 and tm <= 1),
                            stop=(k == KP - 1 and tm in (0, NTERM - 1)),
                            perf_mode=mybir.MatmulPerfMode.DoubleRow,
                        )
            nc.scalar.copy(
                out=em48, in_=pem[:, 0, :].rearrange("p (b t) -> p b t", b=BLOC)
            )
            nc.vector.scalar_tensor_tensor(
                out=em48,
                in0=pem[:, 1, :].rearrange("p (b t) -> p b t", b=BLOC),
                scalar=1.0 / 256.0,
                in1=em48,
                op0=AL.mult,
                op1=AL.add,
            )
            nc.vector.tensor_sub(demf, em48[:, :, 1:], em48[:, :, 0 : T - 1])
            nc.gpsimd.tensor_sub(ndemf, em48[:, :, 0 : T - 1], em48[:, :, 1:])
            epctx.__exit__(None, None, None)  # xt/ef space no longer needed

            # ---- scans ----
            def fwd_step(t):
                p = t + 1
                # off-path bcast term (inputs >= 2 steps old), on Pool
                if p >= 3:
                    q = p % 2
                    nc.gpsimd.tensor_tensor(
                        ndm[:, q, :],
                        demf[:, :, p - 3],
                        mslab[:, :, p - 2],
                        op=AL.subtract,
                    )
                for pair in range(2):
                    ps = pscA[:, pair, :]
                    if p == 1:
                        smm(ps[:, 0:P2], tfr, first=True)
                        bmm(ps[:, 0:P2], d0r, k96=True)
                    elif _fwd_refresh(p, pair):
                        smm(ps[:, 0:P2], tfr, first=True)
                        for sub in range(2):
                            b = 2 * pair + sub
                            blk = ps[:, L * sub : L * sub + L]
                            bmm(blk, em48[:, b, p - 2 : p - 1])
                            bmm(blk, mslab[:, b, p - 1 : p])
                    else:
                        q = p % 2
                        for sub in range(2):
                            b = 2 * pair + sub
                            blk = ps[:, L * sub : L * sub + L]
                            bmm(blk, mslab[:, b, p - 1 : p])
                for pair in range(2):
                    if p == 1 or _fwd_refresh(p, pair):
                        continue
                    ps = pscA[:, pair, :]
                    q = p % 2
                    for sub in range(2):
                        b = 2 * pair + sub
                        blk = ps[:, L * sub : L * sub + L]
                        bmm(blk, ndm[:, q, b : b + 1])
                nc.vector.tensor_reduce(
                    out=mslab[:, :, p : p + 1],
                    in_=pscA[:, :, 0:P2].rearrange("p a (s i) -> p a s i", s=2),
                    axis=AX.X,
                    op=AL.max,
                )

            def bwd_step(t):
                pos = T - 1 - t
                if pos <= T - 2:
                    # nb_pos = -demf[pos] - beta_{pos+2} (off-path, Pool)
                    q = t % 2
                    nc.gpsimd.tensor_tensor(
                        nbt[:, q, :],
                        ndemf[:, :, pos],
                        bslab[:, :, pos + 2],
                        op=AL.subtract,
                    )
                for pair in range(2):
                    pb = psbA[:, pair, :]
                    rf = _bwd_refresh(t, pair)
                    for sub in range(2):
                        b = 2 * pair + sub
                        blk = pb[:, 56 * sub : 56 * sub + L]
                        if rf:
                            nc.tensor.matmul(
                                blk,
                                tbr,
                                id48,
                                start=(sub == 0),
                                stop=False,
                                is_transpose=True,
                                skip_group_check=True,
                            )
                            bmm(blk, em48[:, b, pos : pos + 1])
                            if t > 0:
                                bmm(blk, bslab[:, b, pos + 1 : pos + 2])
                        else:
                            bmm(blk, bslab[:, b, pos + 1 : pos + 2])
                for pair in range(2):
                    if _bwd_refresh(t, pair):
                        continue
                    pb = psbA[:, pair, :]
                    q = t % 2
                    for sub in range(2):
                        b = 2 * pair + sub
                        blk = pb[:, 56 * sub : 56 * sub + L]
                        bmm(blk, nbt[:, q, b : b + 1])
                nc.vector.tensor_reduce(
                    out=bslab[:, :, pos : pos + 1],
                    in_=psbA[:, :, 0:112].rearrange("p a (s i) -> p a s i", i=56)[
                        :, :, :, 0:49
                    ],
                    axis=AX.X,
                    op=AL.max,
                )

            # middle combine-sum columns [C0, C1) become ready mid-loop
            # (mslab fills forward, bslab backward: col n ready at
            # t = max(n-1, T-1-n))
            C0, C1 = 8, 122
            for t in range(T):
                fwd_step(t)
                if t < T - 1:  # beta_0 (t=127) is never consumed
                    bwd_step(t)
                if t == T - 8:
                    nc.gpsimd.tensor_add(
                        dsl[:, :, C0:C1], mslab[:, :, C0:C1], em48[:, :, C0 - 1 : C1 - 1]
                    )
                    nc.gpsimd.tensor_add(
                        fsl[:, :, C0:C1], dsl[:, :, C0:C1], bslab[:, :, C0:C1]
                    )

            # ---- combine ----
            id128f = idb_sb

            def ptrans(out_psum, in_sb):
                nc.tensor.matmul(
                    out_psum,
                    in_sb,
                    id128f[0 : in_sb.shape[0], 0 : in_sb.shape[0]],
                    start=True,
                    stop=True,
                    is_transpose=True,
                    skip_group_check=True,
                )

            # dsl: col0 = d0, cols 1: = m + em;  fsl = dsl + beta
            # (middle columns were computed during the scan loop)
            nc.vector.tensor_add(
                dsl[:, :, 1:C0], mslab[:, :, 1:C0], em48[:, :, 0 : C0 - 1]
            )
            nc.vector.tensor_add(
                dsl[:, :, C1:], mslab[:, :, C1:], em48[:, :, C1 - 1 :]
            )
            nc.vector.tensor_copy(dsl[:, :, 0:1], d0col.broadcast_to([L, BLOC, 1]))
            nc.vector.tensor_add(
                fsl[:, :, 0:C0], dsl[:, :, 0:C0], bslab[:, :, 0:C0]
            )
            nc.vector.tensor_add(
                fsl[:, :, C1:], dsl[:, :, C1:], bslab[:, :, C1 : T + 1]
            )

            # per-b transposes to [T, 48] (fresh PSUM banks) + max/argmax
            pcb = ppcb.tile([128, 2, 512], F32, name="pcb")
            for b in range(BLOC):
                ptd = pcb[:, 0, b * L : (b + 1) * L][0:T, :]
                ptf = pcb[:, 1, b * L : (b + 1) * L][0:T, :]
                ptrans(ptd, dsl[:, b, 1:])
                ptrans(ptf, fsl[:, b, 1:])
            nc.vector.tensor_reduce(
                out=nm,
                in_=pcb[0:T, 0, 0 : BLOC * L].rearrange("p (b l) -> p b l", b=BLOC),
                axis=AX.X,
                op=AL.max,
            )
            for b in range(BLOC):
                ptf = pcb[:, 1, b * L : (b + 1) * L][0:T, :]
                nc.vector.max(fm8[:, b, :], ptf)
                nc.vector.max_index(fi8[:, b, :], fm8[:, b, :], ptf)
            nc.vector.tensor_copy(fidx, fi8[:, :, 0])

            # end_n per b from n_maxs (col 0 of delta_full maxes to 0)
            pnm = pcb[0:BLOC, 1, 192 : 192 + T]
            ptrans(pnm, nm)
            nc.vector.memset(nmb[:, 0:1], 0.0)
            nc.scalar.copy(out=nmb[:, 1:], in_=pnm)
            nc.vector.max(en8, nmb)
            nc.vector.max_index(eni8, en8, nmb)
            nc.vector.tensor_copy(endf, eni8[:, 0:1])

            # active mask act[b, n] = (n <= end_n), transposed to [T, 4]
            nc.vector.tensor_scalar(
                out=act, in0=io129[0:BLOC, :], scalar1=endf, scalar2=None,
                op0=AL.is_le,
            )
            pact = pcb[0:T, 0, 192 : 192 + BLOC]
            ptrans(pact, act[:, 1:])
            nc.scalar.copy(out=actT, in_=pact)

            # y one-hots
            for b in range(BLOC):
                nc.vector.tensor_scalar(
                    out=ybig[:, b, :],
                    in0=io48[0:T, :],
                    scalar1=fidx[:, b : b + 1],
                    scalar2=actT[:, b : b + 1],
                    op0=AL.is_equal,
                    op1=AL.mult,
                )
            nc.sync.dma_start(out=y[:, :, :], in_=ybig)

    nc.finalize()
    _legalize_sync_waits(nc)
    return nc


def _legalize_sync_waits(nc):
    """This container's walrus accepts at most ONE sync wait per instruction.

    Split excess waits onto Drain instructions inserted just before the
    offending instruction (same engine, so the waits still complete before it
    issues; an idle-pipe Drain costs ~12ns).  Applied to the serialized BIR
    only -- CoreSim consumes the in-memory module and is unaffected.
    """
    import json as _json

    m = _json.loads(nc.to_json_bytes())
    for f in m["functions"]:
        for blk in f["blocks"]:
            out = []
            for ins in blk["instructions"]:
                si = ins.get("sync_info") or {}
                w = si.get("on_wait") or []
                if len(w) > 1:
                    for j, wx in enumerate(w[:-1]):
                        out.append(
                            {
                                "debug": ins.get("debug", 0),
                                "engine": ins["engine"],
                                "ins": [],
                                "outs": [],
                                "name": f"{ins['name']}-w{j}",
                                "opcode": "Drain",
                                "sync_info": {"on_update": [], "on_wait": [wx]},
                            }
                        )
                    si["on_wait"] = [w[-1]]
                out.append(ins)
            blk["instructions"] = out
    blob = _json.dumps(m).encode()
    nc.to_json_bytes = lambda: blob


def make_consts():
    f32 = np.float32
    c = np.zeros((128, CW), f32)
    c[0:L, 97] = NEG
    c[0, 97] = 0.0
    d0 = c[0:L, 97].copy()
    c[0:P2, 96] = np.concatenate([d0, d0])
    c[0:L, 98:227] = np.arange(T + 1, dtype=f32)[None, :]
    c[:, 227:275] = np.arange(L, dtype=f32)[None, :]
    return c


def make_in_maps(X, t_feats, e_feats):
    f32 = np.float32
    t_feats = np.asarray(t_feats, dtype=f32)
    e_feats = np.asarray(e_feats, dtype=f32)
    c = make_consts()
    c[0:P2, 0:L] = np.vstack([t_feats, t_feats])
    c[0:L, L : 2 * L] = t_feats.T

    idb = np.eye(128, dtype=f32)

    # e blob [v%128, kp, term, j, L] fp8: 4-term exact-to-~2^-16 split of e^T
    fp8 = mybir.dt.np(FP8)
    eTf = np.zeros((NK * 128, L), f32)
    eTf[:V] = np.ascontiguousarray(e_feats.T)
    terms = []
    t0 = eTf.astype(fp8)
    terms.append(t0)
    rs = (eTf - t0.astype(f32)) * 256.0
    for _ in range(NTERM - 1):
        t = rs.astype(fp8)
        terms.append(t)
        rs = rs - t.astype(f32)
    efm = np.ascontiguousarray(
        np.stack(terms, axis=1)              # [NK*128, NTERM, L]
        .reshape(KP, 2, 128, NTERM, L)       # [kp, j, p, term, L]
        .transpose(2, 0, 3, 1, 4)            # [p, kp, term, j, L]
    )

    # x blob per core [v%128, kp, j, (b t)] in fp8 (one-hot: exact)
    X = np.asarray(X)
    in_maps = []
    for ci in range(NCORES):
        Xc = np.zeros((BLOC, T, NK * 128), f32)
        Xc[:, :, :V] = X[ci * BLOC : (ci + 1) * BLOC]
        # [b, t, kp, j, p] -> [p, kp, j, b, t]
        xb = np.ascontiguousarray(
            Xc.reshape(BLOC, T, KP, 2, 128)
            .transpose(4, 2, 3, 0, 1)
            .reshape(128, KP, 2, BT)
        ).astype(fp8)
        in_maps.append({"x": xb, "eT": efm, "consts": c, "idb": idb})
    return in_maps


_NC = None


def _get_nc():
    global _NC
    if _NC is None:
        _NC = build_nc()
    return _NC


def kernel(X, t_feats, e_feats):
    in_maps = make_in_maps(X, t_feats, e_feats)
    nc = _get_nc()
    res = run_bass_kernel_spmd(nc, in_maps, list(range(NCORES)))
    out = np.concatenate(
        [res.results[ci]["y"].transpose(1, 0, 2) for ci in range(NCORES)], axis=0
    )
    return np.ascontiguousarray(out, dtype=np.float32)


# revision 40
# speedup vs baseline: 1.0069x; 1.0010x over previous
"""CRF Viterbi decode kernel for Trainium2 (Bass), data-parallel over batch.

Problem shapes (hardcoded): X [32,128,10000] f32 one-hot, t_feats [48,48],
e_feats [48,10000].  Output Y_hat [32,128,48] f32 one-hot.

Sharding: batch 32 -> 8 cores x 4.  t_feats / e_feats replicated.

Per-core pipeline (4 batch elems, labels on 48 partitions throughout):
  1. emissions em[l, (b,t)] = e_feats @ X^T: the host pre-transposes the
     one-hot X into an fp8 [v%128, kpair, j, (b t)] blob (0/1 exact in
     fp8) and splits e_feats^T into 4 exact fp8 terms (term 0 unscaled
     into PSUM bank A; terms 1-3 in 2^8-scaled space into bank B;
     combined as A + 2^-8 B).  Emissions are then 160 DoubleRow fp8
     matmuls (2 v-chunks per pass, 0.5 cyc/row) chasing the sliced X
     DMA -- no on-device transposes or staging.
  2. forward Viterbi: per pair of batch elems a PSUM-resident score
     tile psc[j, (sub i)] = t[i,j] + delta[i], updated per step with
     two 48-col f32 broadcast matmuls per elem: on-path bcast(m_{p-1})
     and off-path bcast(demf[p-3] - m_{p-2}) (2-step slack, built on
     the idle Pool engine).  One merged DVE tensor_reduce over both
     pairs' banks produces m_p for all 4 elems.  Periodic staggered
     full refreshes bound fp drift (exact f32 elsewhere).
  3. backward "end-anytime" scan, same structure with t^T, 56-stride
     segments whose permanent zero columns implement beta = max(0, .).
     Runs anti-phased with the forward scan: per ~1us step the two
     chains interleave PE broadcast clusters with DVE reduces.
  4. combine: y_n = onehot(argmax_j delta_n(j)+beta_n(j)) * [n <=
     end_n] via per-elem PE transposes + DVE max/max_index (first-index
     tie semantics match the reference argmax); the middle score
     columns are pre-summed during the scan loop.  Output is written
     label-minor [T, B, L] and unscrambled on the host.

Numerics notes for this stack: float32r matmuls round their inputs on
the NEFF/ucode path (CoreSim does not model it), so every scan matmul
stays plain f32; fp8/bf16 matmul inputs are exact by construction.
GPSIMD cannot touch PSUM, scalar_tensor_tensor does not exist on Pool,
and partition_all_reduce does not compile -- hence the DVE-only
reduces and Pool doing only small SBUF tensor_tensor work.
"""

import os
import sys

import numpy as np

for _p in ("/opt/trn_rl_repo",):
    if _p not in sys.path and os.path.isdir(_p):
        sys.path.insert(0, _p)

import concourse.bass as bass
import concourse.tile as tile
from concourse import mybir
from concourse.bass_utils import run_bass_kernel_spmd

F32 = mybir.dt.float32
F32R = mybir.dt.float32r
BF16 = mybir.dt.bfloat16
FP8 = mybir.dt.float8e4
AL = mybir.AluOpType
AX = mybir.AxisListType

B, T, V, L = 32, 128, 10000, 48
NCORES = 8
BLOC = B // NCORES          # 4 batch elems per core
NK = 80                     # V chunks of 128 (tail zero-padded; even for DoubleRow)
KP = NK // 2                # DoubleRow processes 2 chunks per matmul
NTERM = 4                   # fp8 e-term count (exact split of e_feats)
BT = BLOC * T               # 512 moving columns, b-major
NEG = -1.0e30
P2 = 2 * L                  # 96

# consts layout [128, CW] f32:
#  cols 0:48     tfS   [96,48]  vstack(t, t)        (fwd refresh lhsT)
#  cols 48:96    tbr   [48,48]  t.T                 (bwd refresh lhsT)
#  col  96       d0stack [96,1]
#  col  97       d0col   [48,1]
#  cols 98:227   iota129 [48,129]
#  cols 227:275  iota48  [128,48]
CW = 275

# fwd refresh schedule: always at p==2 (clears the +-1e30 d0 arithmetic),
# then every 16 steps staggered per pair.  bwd staggered likewise on t.
RF = 16


def _fwd_refresh(p, pair):
    if p == 2:
        return True
    return p > 2 and (p - 2 - 8 * pair) % RF == 0


def _bwd_refresh(t, pair):
    # phases 6/10 keep the last refreshes at t=118/122, avoiding a wasted
    # full rebuild on the backward scan's final steps
    if t == 0:
        return True
    return (t - 6 - 4 * pair) % RF == 0


def build_nc():
    nc = bass.Bass()

    x = nc.declare_dram_parameter("x", [128, KP, 2, BT], FP8, isOutput=False)
    eT = nc.declare_dram_parameter("eT", [128, KP, NTERM, 2, L], FP8, isOutput=False)
    consts = nc.declare_dram_parameter("consts", [128, CW], F32, isOutput=False)
    idb = nc.declare_dram_parameter("idb", [128, 128], F32, isOutput=False)
    y = nc.declare_dram_parameter("y", [T, BLOC, L], F32, isOutput=True)

    with tile.TileContext(nc) as tc:
        from contextlib import ExitStack

        with ExitStack() as ctx:
            cons = ctx.enter_context(tc.tile_pool(name="cons", bufs=1))
            pers = ctx.enter_context(tc.tile_pool(name="pers", bufs=1))
            ppem = ctx.enter_context(tc.tile_pool(name="ppem", bufs=1, space="PSUM"))
            ppfw = ctx.enter_context(tc.tile_pool(name="ppfw", bufs=1, space="PSUM"))
            ppbw = ctx.enter_context(tc.tile_pool(name="ppbw", bufs=1, space="PSUM"))
            ppcb = ctx.enter_context(tc.tile_pool(name="ppcb", bufs=1, space="PSUM"))

            # ---- constants ----
            cons_sb = cons.tile([128, CW], F32)
            nc.sync.dma_start(out=cons_sb, in_=consts[:, :])
            d0col = cons_sb[0:L, 97:98]
            io129 = cons_sb[0:L, 98:227]
            io48 = cons_sb[:, 227:275]

            tfr = cons_sb[0:P2, 0:L]
            tbr = cons_sb[0:L, L : 2 * L]
            d0r = cons_sb[0:P2, 96:97]

            idb_sb = cons.tile([128, 128], F32)
            nc.sync.dma_start(out=idb_sb, in_=idb[:, :])
            id96 = idb_sb[0:P2, 0:P2]
            id48 = idb_sb[0:L, 0:L]

            # ---- persistent state ----
            em48 = pers.tile([L, BLOC, T], F32, name="em48")
            demf = pers.tile([L, BLOC, T - 1], F32, name="demf")
            ndemf = pers.tile([L, BLOC, T - 1], F32, name="ndemf")
            mslab = pers.tile([L, BLOC, T + 1], F32, name="mslab")
            bslab = pers.tile([L, BLOC, T + 2], F32, name="bslab")
            ndm = pers.tile([L, 2, BLOC], F32, name="ndm")
            nbt = pers.tile([L, 2, BLOC], F32, name="nbt")
            dsl = pers.tile([L, BLOC, T + 1], F32, name="dsl")
            fsl = pers.tile([L, BLOC, T + 1], F32, name="fsl")
            fi8 = pers.tile([T, BLOC, 8], mybir.dt.uint32, name="fi8")
            fm8 = pers.tile([T, BLOC, 8], F32, name="fm8")
            fidx = pers.tile([T, BLOC], F32, name="fidx")
            nm = pers.tile([T, BLOC], F32, name="nm")
            nmb = pers.tile([BLOC, T + 1], F32, name="nmb")
            en8 = pers.tile([BLOC, 8], F32, name="en8")
            eni8 = pers.tile([BLOC, 8], mybir.dt.uint32, name="eni8")
            endf = pers.tile([BLOC, 1], F32, name="endf")
            act = pers.tile([BLOC, T + 1], F32, name="act")
            actT = pers.tile([T, BLOC], F32, name="actT")
            ybig = pers.tile([T, BLOC, L], F32, name="ybig")

            pscA = ppfw.tile([L, 2, 512], F32, name="pscA")
            psbA = ppbw.tile([L, 2, 512], F32, name="psbA")
            pem = ppem.tile([L, 2, 512], F32, name="pem")
            # bwd uses 49-wide segments; cols 48/97 stay 0 forever (the
            # max-with-zero floor).  beta_T = 0 (bslab col T).
            nc.vector.memset(psbA[:, :, 48:105:56], 0.0)
            nc.vector.memset(bslab[:, :, T : T + 1], 0.0)
            nc.vector.memset(bslab[:, :, 0:1], 0.0)

            def bmm(out, col, first=False, last=False, k96=False):
                """Broadcast col ([48,1] or [96,1]) along the free dim of out.

                lhsT/out are bitcast to f32r (same bits as f32 here) so the
                bf16 identity rhs keys the 1.0 cyc/row transpose path.
                """
                kk = P2 if k96 else L
                nc.tensor.matmul(
                    out,
                    col.broadcast_to([kk, L]),
                    id96 if k96 else id48,
                    start=first,
                    stop=last,
                    is_transpose=True,
                    skip_group_check=True,
                )

            def smm(out, lhsT, first=False):
                """Static 96-wide refresh matmul (t-term)."""
                nc.tensor.matmul(
                    out,
                    lhsT,
                    id96,
                    start=first,
                    stop=False,
                    is_transpose=True,
                    skip_group_check=True,
                )

            # ---- emissions: xt/ef live only in this phase; their pool is
            # closed afterwards so the beta slab can reuse the space ----
            epctx = tc.tile_pool(name="ep", bufs=1)
            ep = epctx.__enter__()
            ef = ep.tile([128, KP, NTERM, 2, L], FP8)
            xt = ep.tile([128, KP, 2, BT], FP8)
            KSL = [0, 5, 10, 15, 20, 25, 30, 35, KP]
            for s in range(len(KSL) - 1):
                k0, k1 = KSL[s], KSL[s + 1]
                nc.sync.dma_start(
                    out=ef[:, k0:k1, :, :, :], in_=eT[:, k0:k1, :, :, :]
                )
                nc.sync.dma_start(out=xt[:, k0:k1, :, :], in_=x[:, k0:k1, :, :])
                for k in range(k0, k1):
                    for tm in range(NTERM):
                        # term 0 -> bank A (unscaled); terms 1-3 -> bank B
                        # (stored x256; the combine scales by 2^-8)
                        g = 0 if tm == 0 else 1
                        nc.tensor.matmul(
                            pem[:, g, :],
                            ef[:, k, tm, :, :],
                            xt[:, k, :, :],
                            start=(k == 0 and tm <= 1),
                            stop=(k == KP - 1 and tm in (0, NTERM - 1)),
                            perf_mode=mybir.MatmulPerfMode.DoubleRow,
                        )
            nc.scalar.copy(
                out=em48, in_=pem[:, 0, :].rearrange("p (b t) -> p b t", b=BLOC)
            )
            nc.vector.scalar_tensor_tensor(
                out=em48,
                in0=pem[:, 1, :].rearrange("p (b t) -> p b t", b=BLOC),
                scalar=1.0 / 256.0,
                in1=em48,
                op0=AL.mult,
                op1=AL.add,
            )
            nc.vector.tensor_sub(demf, em48[:, :, 1:], em48[:, :, 0 : T - 1])
            nc.gpsimd.tensor_sub(ndemf, em48[:, :, 0 : T - 1], em48[:, :, 1:])
            epctx.__exit__(None, None, None)  # xt/ef space no longer needed

            # ---- scans ----
            def fwd_step(t):
                p = t + 1
                # off-path bcast term (inputs >= 2 steps old), on Pool
                if p >= 3:
                    q = p % 2
                    nc.gpsimd.tensor_tensor(
                        ndm[:, q, :],
                        demf[:, :, p - 3],
                        mslab[:, :, p - 2],
                        op=AL.subtract,
                    )
                for pair in range(2):
                    ps = pscA[:, pair, :]
                    if p == 1:
                        smm(ps[:, 0:P2], tfr, first=True)
                        bmm(ps[:, 0:P2], d0r, k96=True)
                    elif _fwd_refresh(p, pair):
                        smm(ps[:, 0:P2], tfr, first=True)
                        for sub in range(2):
                            b = 2 * pair + sub
                            blk = ps[:, L * sub : L * sub + L]
                            bmm(blk, em48[:, b, p - 2 : p - 1])
                            bmm(blk, mslab[:, b, p - 1 : p])
                    else:
                        q = p % 2
                        for sub in range(2):
                            b = 2 * pair + sub
                            blk = ps[:, L * sub : L * sub + L]
                            bmm(blk, mslab[:, b, p - 1 : p])
                for pair in range(2):
                    if p == 1 or _fwd_refresh(p, pair):
                        continue
                    ps = pscA[:, pair, :]
                    q = p % 2
                    for sub in range(2):
                        b = 2 * pair + sub
                        blk = ps[:, L * sub : L * sub + L]
                        bmm(blk, ndm[:, q, b : b + 1])
                nc.vector.tensor_reduce(
                    out=mslab[:, :, p : p + 1],
                    in_=pscA[:, :, 0:P2].rearrange("p a (s i) -> p a s i", s=2),
                    axis=AX.X,
                    op=AL.max,
                )

            def bwd_step(t):
                pos = T - 1 - t
                if pos <= T - 2:
                    # nb_pos = -demf[pos] - beta_{pos+2} (off-path, Pool)
                    q = t % 2
                    nc.gpsimd.tensor_tensor(
                        nbt[:, q, :],
                        ndemf[:, :, pos],
                        bslab[:, :, pos + 2],
                        op=AL.subtract,
                    )
                for pair in range(2):
                    pb = psbA[:, pair, :]
                    rf = _bwd_refresh(t, pair)
                    for sub in range(2):
                        b = 2 * pair + sub
                        blk = pb[:, 56 * sub : 56 * sub + L]
                        if rf:
                            nc.tensor.matmul(
                                blk,
                                tbr,
                                id48,
                                start=(sub == 0),
                                stop=False,
                                is_transpose=True,
                                skip_group_check=True,
                            )
                            bmm(blk, em48[:, b, pos : pos + 1])
                            if t > 0:
                                bmm(blk, bslab[:, b, pos + 1 : pos + 2])
                        else:
                            bmm(blk, bslab[:, b, pos + 1 : pos + 2])
                for pair in range(2):
                    if _bwd_refresh(t, pair):
                        continue
                    pb = psbA[:, pair, :]
                    q = t % 2
                    for sub in range(2):
                        b = 2 * pair + sub
                        blk = pb[:, 56 * sub : 56 * sub + L]
                        bmm(blk, nbt[:, q, b : b + 1])
                nc.vector.tensor_reduce(
                    out=bslab[:, :, pos : pos + 1],
                    in_=psbA[:, :, 0:112].rearrange("p a (s i) -> p a s i", i=56)[
                        :, :, :, 0:49
                    ],
                    axis=AX.X,
                    op=AL.max,
                )

            # middle combine-sum columns [C0, C1) become ready mid-loop
            # (mslab fills forward, bslab backward: col n ready at
            # t = max(n-1, T-1-n))
            C0, C1 = 8, 122
            for t in range(T):
                fwd_step(t)
                if t < T - 1:  # beta_0 (t=127) is never consumed
                    bwd_step(t)
                if t == T - 8:
                    nc.gpsimd.tensor_add(
                        dsl[:, :, C0:C1], mslab[:, :, C0:C1], em48[:, :, C0 - 1 : C1 - 1]
                    )
                    nc.gpsimd.tensor_add(
                        fsl[:, :, C0:C1], dsl[:, :, C0:C1], bslab[:, :, C0:C1]
                    )

            # ---- combine ----
            id128f = idb_sb

            def ptrans(out_psum, in_sb):
                nc.tensor.matmul(
                    out_psum,
                    in_sb,
                    id128f[0 : in_sb.shape[0], 0 : in_sb.shape[0]],
                    start=True,
                    stop=True,
                    is_transpose=True,
                    skip_group_check=True,
                )

            # dsl: col0 = d0, cols 1: = m + em;  fsl = dsl + beta
            # (middle columns were computed during the scan loop)
            nc.vector.tensor_add(
                dsl[:, :, 1:C0], mslab[:, :, 1:C0], em48[:, :, 0 : C0 - 1]
            )
            nc.vector.tensor_add(
                dsl[:, :, C1:], mslab[:, :, C1:], em48[:, :, C1 - 1 :]
            )
            nc.vector.tensor_copy(dsl[:, :, 0:1], d0col.broadcast_to([L, BLOC, 1]))
            nc.vector.tensor_add(
                fsl[:, :, 0:C0], dsl[:, :, 0:C0], bslab[:, :, 0:C0]
            )
            nc.vector.tensor_add(
                fsl[:, :, C1:], dsl[:, :, C1:], bslab[:, :, C1 : T + 1]
            )

            # per-b transposes to [T, 48] (fresh PSUM banks) + max/argmax
            pcb = ppcb.tile([128, 2, 512], F32, name="pcb")
            for b in range(BLOC):
                ptd = pcb[:, 0, b * L : (b + 1) * L][0:T, :]
                ptf = pcb[:, 1, b * L : (b + 1) * L][0:T, :]
                ptrans(ptd, dsl[:, b, 1:])
                ptrans(ptf, fsl[:, b, 1:])
            nc.vector.tensor_reduce(
                out=nm,
                in_=pcb[0:T, 0, 0 : BLOC * L].rearrange("p (b l) -> p b l", b=BLOC),
                axis=AX.X,
                op=AL.max,
            )
            for b in range(BLOC):
                ptf = pcb[:, 1, b * L : (b + 1) * L][0:T, :]
                nc.vector.max(fm8[:, b, :], ptf)
                nc.vector.max_index(fi8[:, b, :], fm8[:, b, :], ptf)
            nc.vector.tensor_copy(fidx, fi8[:, :, 0])

            # end_n per b from n_maxs (col 0 of delta_full maxes to 0)
            pnm = pcb[0:BLOC, 1, 192 : 192 + T]
            ptrans(pnm, nm)
            nc.vector.memset(nmb[:, 0:1], 0.0)
            nc.scalar.copy(out=nmb[:, 1:], in_=pnm)
            nc.vector.max(en8, nmb)
            nc.vector.max_index(eni8, en8, nmb)
            nc.vector.tensor_copy(endf, eni8[:, 0:1])

            # active mask act[b, n] = (n <= end_n), transposed to [T, 4]
            nc.vector.tensor_scalar(
                out=act, in0=io129[0:BLOC, :], scalar1=endf, scalar2=None,
                op0=AL.is_le,
            )
            pact = pcb[0:T, 0, 192 : 192 + BLOC]
            ptrans(pact, act[:, 1:])
            nc.scalar.copy(out=actT, in_=pact)

            # y one-hots
            for b in range(BLOC):
                nc.vector.tensor_scalar(
                    out=ybig[:, b, :],
                    in0=io48[0:T, :],
                    scalar1=fidx[:, b : b + 1],
                    scalar2=actT[:, b : b + 1],
                    op0=AL.is_equal,
                    op1=AL.mult,
                )
            nc.sync.dma_start(out=y[:, :, :], in_=ybig)

    nc.finalize()
    _legalize_sync_waits(nc)
    return nc


def _legalize_sync_waits(nc):
    """This container's walrus accepts at most ONE sync wait per instruction.

    Split excess waits onto Drain instructions inserted just before the
    offending instruction (same engine, so the waits still complete before it
    issues; an idle-pipe Drain costs ~12ns).  Applied to the serialized BIR
    only -- CoreSim consumes the in-memory module and is unaffected.
    """
    import json as _json

    m = _json.loads(nc.to_json_bytes())
    for f in m["functions"]:
        for blk in f["blocks"]:
            out = []
            for ins in blk["instructions"]:
                si = ins.get("sync_info") or {}
                w = si.get("on_wait") or []
                if len(w) > 1:
                    for j, wx in enumerate(w[:-1]):
                        out.append(
                            {
                                "debug": ins.get("debug", 0),
                                "engine": ins["engine"],
                                "ins": [],
                                "outs": [],
                                "name": f"{ins['name']}-w{j}",
                                "opcode": "Drain",
                                "sync_info": {"on_update": [], "on_wait": [wx]},
                            }
                        )
                    si["on_wait"] = [w[-1]]
                out.append(ins)
            blk["instructions"] = out
    blob = _json.dumps(m).encode()
    nc.to_json_bytes = lambda: blob


def make_consts():
    f32 = np.float32
    c = np.zeros((128, CW), f32)
    c[0:L, 97] = NEG
    c[0, 97] = 0.0
    d0 = c[0:L, 97].copy()
    c[0:P2, 96] = np.concatenate([d0, d0])
    c[0:L, 98:227] = np.arange(T + 1, dtype=f32)[None, :]
    c[:, 227:275] = np.arange(L, dtype=f32)[None, :]
    return c


def make_in_maps(X, t_feats, e_feats):
    f32 = np.float32
    t_feats = np.asarray(t_feats, dtype=f32)
    e_feats = np.asarray(e_feats, dtype=f32)
    c = make_consts()
    c[0:P2, 0:L] = np.vstack([t_feats, t_feats])
    c[0:L, L : 2 * L] = t_feats.T

    idb = np.eye(128, dtype=f32)

    # e blob [v%128, kp, term, j, L] fp8: 4-term exact-to-~2^-16 split of e^T
    fp8 = mybir.dt.np(FP8)
    eTf = np.zeros((NK * 128, L), f32)
    eTf[:V] = np.ascontiguousarray(e_feats.T)
    terms = []
    t0 = eTf.astype(fp8)
    terms.append(t0)
    rs = (eTf - t0.astype(f32)) * 256.0
    for _ in range(NTERM - 1):
        t = rs.astype(fp8)
        terms.append(t)
        rs = rs - t.astype(f32)
    efm = np.ascontiguousarray(
        np.stack(terms, axis=1)              # [NK*128, NTERM, L]
        .reshape(KP, 2, 128, NTERM, L)       # [kp, j, p, term, L]
        .transpose(2, 0, 3, 1, 4)            # [p, kp, term, j, L]
    )

    # x blob per core [v%128, kp, j, (b t)] in fp8 (one-hot: exact)
    X = np.asarray(X)
    in_maps = []
    for ci in range(NCORES):
        Xc = np.zeros((BLOC, T, NK * 128), f32)
        Xc[:, :, :V] = X[ci * BLOC : (ci + 1) * BLOC]
        # [b, t, kp, j, p] -> [p, kp, j, b, t]
        xb = np.ascontiguousarray(
            Xc.reshape(BLOC, T, KP, 2, 128)
            .transpose(4, 2, 3, 0, 1)
            .reshape(128, KP, 2, BT)
        ).astype(fp8)
        in_maps.append({"x": xb, "eT": efm, "consts": c, "idb": idb})
    return in_maps


_NC = None


def _get_nc():
    global _NC
    if _NC is None:
        _NC = build_nc()
    return _NC


def kernel(X, t_feats, e_feats):
    in_maps = make_in_maps(X, t_feats, e_feats)
    nc = _get_nc()
    res = run_bass_kernel_spmd(nc, in_maps, list(range(NCORES)))
    out = np.concatenate(
        [res.results[ci]["y"].transpose(1, 0, 2) for ci in range(NCORES)], axis=0
    )
    return np.ascontiguousarray(out, dtype=np.float32)


# revision 41
# speedup vs baseline: 1.0113x; 1.0044x over previous
"""CRF Viterbi decode kernel for Trainium2 (Bass), data-parallel over batch.

Problem shapes (hardcoded): X [32,128,10000] f32 one-hot, t_feats [48,48],
e_feats [48,10000].  Output Y_hat [32,128,48] f32 one-hot.

Sharding: batch 32 -> 8 cores x 4.  t_feats / e_feats replicated.

Per-core pipeline (4 batch elems, labels on 48 partitions throughout):
  1. emissions em[l, (b,t)] = e_feats @ X^T: the host pre-transposes the
     one-hot X into an fp8 [v%128, kpair, j, (b t)] blob (0/1 exact in
     fp8) and splits e_feats^T into 4 exact fp8 terms (term 0 unscaled
     into PSUM bank A; terms 1-3 in 2^8-scaled space into bank B;
     combined as A + 2^-8 B).  Emissions are then 160 DoubleRow fp8
     matmuls (2 v-chunks per pass, 0.5 cyc/row) chasing the sliced X
     DMA -- no on-device transposes or staging.
  2. forward Viterbi: per pair of batch elems a PSUM-resident score
     tile psc[j, (sub i)] = t[i,j] + delta[i], updated per step with
     two 48-col f32 broadcast matmuls per elem: on-path bcast(m_{p-1})
     and off-path bcast(demf[p-3] - m_{p-2}) (2-step slack, built on
     the idle Pool engine).  One merged DVE tensor_reduce over both
     pairs' banks produces m_p for all 4 elems.  Periodic staggered
     full refreshes bound fp drift (exact f32 elsewhere).
  3. backward "end-anytime" scan, same structure with t^T, 56-stride
     segments whose permanent zero columns implement beta = max(0, .).
     Runs anti-phased with the forward scan: per ~1us step the two
     chains interleave PE broadcast clusters with DVE reduces.
  4. combine: y_n = onehot(argmax_j delta_n(j)+beta_n(j)) * [n <=
     end_n] via per-elem PE transposes + DVE max/max_index (first-index
     tie semantics match the reference argmax); the middle score
     columns are pre-summed during the scan loop.  Output is written
     label-minor [T, B, L] and unscrambled on the host.

Numerics notes for this stack: float32r matmuls round their inputs on
the NEFF/ucode path (CoreSim does not model it), so every scan matmul
stays plain f32; fp8/bf16 matmul inputs are exact by construction.
GPSIMD cannot touch PSUM, scalar_tensor_tensor does not exist on Pool,
and partition_all_reduce does not compile -- hence the DVE-only
reduces and Pool doing only small SBUF tensor_tensor work.
"""

import os
import sys

import numpy as np

for _p in ("/opt/trn_rl_repo",):
    if _p not in sys.path and os.path.isdir(_p):
        sys.path.insert(0, _p)

import concourse.bass as bass
import concourse.tile as tile
from concourse import mybir
from concourse.bass_utils import run_bass_kernel_spmd

F32 = mybir.dt.float32
F32R = mybir.dt.float32r
BF16 = mybir.dt.bfloat16
FP8 = mybir.dt.float8e4
AL = mybir.AluOpType
AX = mybir.AxisListType

B, T, V, L = 32, 128, 10000, 48
NCORES = 8
BLOC = B // NCORES          # 4 batch elems per core
NK = 80                     # V chunks of 128 (tail zero-padded; even for DoubleRow)
KP = NK // 2                # DoubleRow processes 2 chunks per matmul
NTERM = 4                   # fp8 e-term count (exact split of e_feats)
BT = BLOC * T               # 512 moving columns, b-major
NEG = -1.0e30
P2 = 2 * L                  # 96

# consts layout [128, CW] f32:
#  cols 0:48     tfS   [96,48]  vstack(t, t)        (fwd refresh lhsT)
#  cols 48:96    tbr   [48,48]  t.T                 (bwd refresh lhsT)
#  col  96       d0stack [96,1]
#  col  97       d0col   [48,1]
#  cols 98:227   iota129 [48,129]
#  cols 227:275  iota48  [128,48]
CW = 275

# fwd refresh schedule: always at p==2 (clears the +-1e30 d0 arithmetic),
# then every 16 steps staggered per pair.  bwd staggered likewise on t.
RF = 16


def _fwd_refresh(p, pair):
    if p == 2:
        return True
    return p > 2 and (p - 2 - 8 * pair) % RF == 0


def _bwd_refresh(t, pair):
    # phases 6/10 keep the last refreshes at t=118/122, avoiding a wasted
    # full rebuild on the backward scan's final steps
    if t == 0:
        return True
    return (t - 6 - 4 * pair) % RF == 0


def build_nc():
    nc = bass.Bass()

    x = nc.declare_dram_parameter("x", [128, KP, 2, BT], FP8, isOutput=False)
    eT = nc.declare_dram_parameter("eT", [128, KP, NTERM, 2, L], FP8, isOutput=False)
    consts = nc.declare_dram_parameter("consts", [128, CW], F32, isOutput=False)
    idb = nc.declare_dram_parameter("idb", [128, 128], F32, isOutput=False)
    y = nc.declare_dram_parameter("y", [T, BLOC, L], F32, isOutput=True)

    with tile.TileContext(nc) as tc:
        from contextlib import ExitStack

        with ExitStack() as ctx:
            cons = ctx.enter_context(tc.tile_pool(name="cons", bufs=1))
            pers = ctx.enter_context(tc.tile_pool(name="pers", bufs=1))
            ppem = ctx.enter_context(tc.tile_pool(name="ppem", bufs=1, space="PSUM"))
            ppfw = ctx.enter_context(tc.tile_pool(name="ppfw", bufs=1, space="PSUM"))
            ppbw = ctx.enter_context(tc.tile_pool(name="ppbw", bufs=1, space="PSUM"))
            ppcb = ctx.enter_context(tc.tile_pool(name="ppcb", bufs=1, space="PSUM"))

            # ---- constants (DMAs issued after the emission stream) ----
            cons_sb = cons.tile([128, CW], F32)
            d0col = cons_sb[0:L, 97:98]
            io129 = cons_sb[0:L, 98:227]
            io48 = cons_sb[:, 227:275]

            tfr = cons_sb[0:P2, 0:L]
            tbr = cons_sb[0:L, L : 2 * L]
            d0r = cons_sb[0:P2, 96:97]

            idb_sb = cons.tile([128, 128], F32)
            id96 = idb_sb[0:P2, 0:P2]
            id48 = idb_sb[0:L, 0:L]

            # ---- persistent state ----
            em48 = pers.tile([L, BLOC, T], F32, name="em48")
            demf = pers.tile([L, BLOC, T - 1], F32, name="demf")
            ndemf = pers.tile([L, BLOC, T - 1], F32, name="ndemf")
            mslab = pers.tile([L, BLOC, T + 1], F32, name="mslab")
            bslab = pers.tile([L, BLOC, T + 2], F32, name="bslab")
            ndm = pers.tile([L, 2, BLOC], F32, name="ndm")
            nbt = pers.tile([L, 2, BLOC], F32, name="nbt")
            dsl = pers.tile([L, BLOC, T + 1], F32, name="dsl")
            fsl = pers.tile([L, BLOC, T + 1], F32, name="fsl")
            fi8 = pers.tile([T, BLOC, 8], mybir.dt.uint32, name="fi8")
            fm8 = pers.tile([T, BLOC, 8], F32, name="fm8")
            fidx = pers.tile([T, BLOC], F32, name="fidx")
            nm = pers.tile([T, BLOC], F32, name="nm")
            nmb = pers.tile([BLOC, T + 1], F32, name="nmb")
            en8 = pers.tile([BLOC, 8], F32, name="en8")
            eni8 = pers.tile([BLOC, 8], mybir.dt.uint32, name="eni8")
            endf = pers.tile([BLOC, 1], F32, name="endf")
            act = pers.tile([BLOC, T + 1], F32, name="act")
            actT = pers.tile([T, BLOC], F32, name="actT")
            ybig = pers.tile([T, BLOC, L], F32, name="ybig")

            pscA = ppfw.tile([L, 2, 512], F32, name="pscA")
            psbA = ppbw.tile([L, 2, 512], F32, name="psbA")
            pem = ppem.tile([L, 2, 512], F32, name="pem")
            # bwd uses 49-wide segments; cols 48/97 stay 0 forever (the
            # max-with-zero floor).  beta_T = 0 (bslab col T).
            nc.vector.memset(psbA[:, :, 48:105:56], 0.0)
            nc.vector.memset(bslab[:, :, T : T + 1], 0.0)
            nc.vector.memset(bslab[:, :, 0:1], 0.0)

            def bmm(out, col, first=False, last=False, k96=False):
                """Broadcast col ([48,1] or [96,1]) along the free dim of out.

                lhsT/out are bitcast to f32r (same bits as f32 here) so the
                bf16 identity rhs keys the 1.0 cyc/row transpose path.
                """
                kk = P2 if k96 else L
                nc.tensor.matmul(
                    out,
                    col.broadcast_to([kk, L]),
                    id96 if k96 else id48,
                    start=first,
                    stop=last,
                    is_transpose=True,
                    skip_group_check=True,
                )

            def smm(out, lhsT, first=False):
                """Static 96-wide refresh matmul (t-term)."""
                nc.tensor.matmul(
                    out,
                    lhsT,
                    id96,
                    start=first,
                    stop=False,
                    is_transpose=True,
                    skip_group_check=True,
                )

            # ---- emissions: xt/ef live only in this phase; their pool is
            # closed afterwards so the beta slab can reuse the space ----
            epctx = tc.tile_pool(name="ep", bufs=1)
            ep = epctx.__enter__()
            ef = ep.tile([128, KP, NTERM, 2, L], FP8)
            xt = ep.tile([128, KP, 2, BT], FP8)
            KSL = [0, 5, 10, 15, 20, 25, 30, 35, KP]
            for s in range(len(KSL) - 1):
                k0, k1 = KSL[s], KSL[s + 1]
                nc.sync.dma_start(
                    out=ef[:, k0:k1, :, :, :], in_=eT[:, k0:k1, :, :, :]
                )
                nc.sync.dma_start(out=xt[:, k0:k1, :, :], in_=x[:, k0:k1, :, :])
                if s == 0:
                    # scan/combine constants ride behind the first slices
                    nc.sync.dma_start(out=cons_sb, in_=consts[:, :])
                    nc.sync.dma_start(out=idb_sb, in_=idb[:, :])
                for k in range(k0, k1):
                    for tm in range(NTERM):
                        # term 0 -> bank A (unscaled); terms 1-3 -> bank B
                        # (stored x256; the combine scales by 2^-8)
                        g = 0 if tm == 0 else 1
                        nc.tensor.matmul(
                            pem[:, g, :],
                            ef[:, k, tm, :, :],
                            xt[:, k, :, :],
                            start=(k == 0 and tm <= 1),
                            stop=(k == KP - 1 and tm in (0, NTERM - 1)),
                            perf_mode=mybir.MatmulPerfMode.DoubleRow,
                        )
            nc.scalar.copy(
                out=em48, in_=pem[:, 0, :].rearrange("p (b t) -> p b t", b=BLOC)
            )
            nc.vector.scalar_tensor_tensor(
                out=em48,
                in0=pem[:, 1, :].rearrange("p (b t) -> p b t", b=BLOC),
                scalar=1.0 / 256.0,
                in1=em48,
                op0=AL.mult,
                op1=AL.add,
            )
            nc.vector.tensor_sub(demf, em48[:, :, 1:], em48[:, :, 0 : T - 1])
            nc.gpsimd.tensor_sub(ndemf, em48[:, :, 0 : T - 1], em48[:, :, 1:])
            epctx.__exit__(None, None, None)  # xt/ef space no longer needed

            # ---- scans ----
            def fwd_step(t):
                p = t + 1
                # off-path bcast term (inputs >= 2 steps old), on Pool
                if p >= 3:
                    q = p % 2
                    nc.gpsimd.tensor_tensor(
                        ndm[:, q, :],
                        demf[:, :, p - 3],
                        mslab[:, :, p - 2],
                        op=AL.subtract,
                    )
                for pair in range(2):
                    ps = pscA[:, pair, :]
                    if p == 1:
                        smm(ps[:, 0:P2], tfr, first=True)
                        bmm(ps[:, 0:P2], d0r, k96=True)
                    elif _fwd_refresh(p, pair):
                        smm(ps[:, 0:P2], tfr, first=True)
                        for sub in range(2):
                            b = 2 * pair + sub
                            blk = ps[:, L * sub : L * sub + L]
                            bmm(blk, em48[:, b, p - 2 : p - 1])
                            bmm(blk, mslab[:, b, p - 1 : p])
                    else:
                        q = p % 2
                        for sub in range(2):
                            b = 2 * pair + sub
                            blk = ps[:, L * sub : L * sub + L]
                            bmm(blk, mslab[:, b, p - 1 : p])
                for pair in range(2):
                    if p == 1 or _fwd_refresh(p, pair):
                        continue
                    ps = pscA[:, pair, :]
                    q = p % 2
                    for sub in range(2):
                        b = 2 * pair + sub
                        blk = ps[:, L * sub : L * sub + L]
                        bmm(blk, ndm[:, q, b : b + 1])
                nc.vector.tensor_reduce(
                    out=mslab[:, :, p : p + 1],
                    in_=pscA[:, :, 0:P2].rearrange("p a (s i) -> p a s i", s=2),
                    axis=AX.X,
                    op=AL.max,
                )

            def bwd_step(t):
                pos = T - 1 - t
                if pos <= T - 2:
                    # nb_pos = -demf[pos] - beta_{pos+2} (off-path, Pool)
                    q = t % 2
                    nc.gpsimd.tensor_tensor(
                        nbt[:, q, :],
                        ndemf[:, :, pos],
                        bslab[:, :, pos + 2],
                        op=AL.subtract,
                    )
                for pair in range(2):
                    pb = psbA[:, pair, :]
                    rf = _bwd_refresh(t, pair)
                    for sub in range(2):
                        b = 2 * pair + sub
                        blk = pb[:, 56 * sub : 56 * sub + L]
                        if rf:
                            nc.tensor.matmul(
                                blk,
                                tbr,
                                id48,
                                start=(sub == 0),
                                stop=False,
                                is_transpose=True,
                                skip_group_check=True,
                            )
                            bmm(blk, em48[:, b, pos : pos + 1])
                            if t > 0:
                                bmm(blk, bslab[:, b, pos + 1 : pos + 2])
                        else:
                            bmm(blk, bslab[:, b, pos + 1 : pos + 2])
                for pair in range(2):
                    if _bwd_refresh(t, pair):
                        continue
                    pb = psbA[:, pair, :]
                    q = t % 2
                    for sub in range(2):
                        b = 2 * pair + sub
                        blk = pb[:, 56 * sub : 56 * sub + L]
                        bmm(blk, nbt[:, q, b : b + 1])
                nc.vector.tensor_reduce(
                    out=bslab[:, :, pos : pos + 1],
                    in_=psbA[:, :, 0:112].rearrange("p a (s i) -> p a s i", i=56)[
                        :, :, :, 0:49
                    ],
                    axis=AX.X,
                    op=AL.max,
                )

            # middle combine-sum columns [C0, C1) become ready mid-loop
            # (mslab fills forward, bslab backward: col n ready at
            # t = max(n-1, T-1-n))
            C0, C1 = 8, 122
            for t in range(T):
                fwd_step(t)
                if t < T - 1:  # beta_0 (t=127) is never consumed
                    bwd_step(t)
                if t == T - 8:
                    nc.gpsimd.tensor_add(
                        dsl[:, :, C0:C1], mslab[:, :, C0:C1], em48[:, :, C0 - 1 : C1 - 1]
                    )
                    nc.gpsimd.tensor_add(
                        fsl[:, :, C0:C1], dsl[:, :, C0:C1], bslab[:, :, C0:C1]
                    )

            # ---- combine ----
            id128f = idb_sb

            def ptrans(out_psum, in_sb):
                nc.tensor.matmul(
                    out_psum,
                    in_sb,
                    id128f[0 : in_sb.shape[0], 0 : in_sb.shape[0]],
                    start=True,
                    stop=True,
                    is_transpose=True,
                    skip_group_check=True,
                )

            # dsl: col0 = d0, cols 1: = m + em;  fsl = dsl + beta
            # (middle columns were computed during the scan loop)
            nc.vector.tensor_add(
                dsl[:, :, 1:C0], mslab[:, :, 1:C0], em48[:, :, 0 : C0 - 1]
            )
            nc.vector.tensor_add(
                dsl[:, :, C1:], mslab[:, :, C1:], em48[:, :, C1 - 1 :]
            )
            nc.vector.tensor_copy(dsl[:, :, 0:1], d0col.broadcast_to([L, BLOC, 1]))
            nc.vector.tensor_add(
                fsl[:, :, 0:C0], dsl[:, :, 0:C0], bslab[:, :, 0:C0]
            )
            nc.vector.tensor_add(
                fsl[:, :, C1:], dsl[:, :, C1:], bslab[:, :, C1 : T + 1]
            )

            # per-b transposes to [T, 48] (fresh PSUM banks) + max/argmax
            pcb = ppcb.tile([128, 2, 512], F32, name="pcb")
            for b in range(BLOC):
                ptd = pcb[:, 0, b * L : (b + 1) * L][0:T, :]
                ptf = pcb[:, 1, b * L : (b + 1) * L][0:T, :]
                ptrans(ptd, dsl[:, b, 1:])
                ptrans(ptf, fsl[:, b, 1:])
            nc.vector.tensor_reduce(
                out=nm,
                in_=pcb[0:T, 0, 0 : BLOC * L].rearrange("p (b l) -> p b l", b=BLOC),
                axis=AX.X,
                op=AL.max,
            )
            for b in range(BLOC):
                ptf = pcb[:, 1, b * L : (b + 1) * L][0:T, :]
                nc.vector.max(fm8[:, b, :], ptf)
                nc.vector.max_index(fi8[:, b, :], fm8[:, b, :], ptf)
            nc.vector.tensor_copy(fidx, fi8[:, :, 0])

            # end_n per b from n_maxs (col 0 of delta_full maxes to 0)
            pnm = pcb[0:BLOC, 1, 192 : 192 + T]
            ptrans(pnm, nm)
            nc.vector.memset(nmb[:, 0:1], 0.0)
            nc.scalar.copy(out=nmb[:, 1:], in_=pnm)
            nc.vector.max(en8, nmb)
            nc.vector.max_index(eni8, en8, nmb)
            nc.vector.tensor_copy(endf, eni8[:, 0:1])

            # active mask act[b, n] = (n <= end_n), transposed to [T, 4]
            nc.vector.tensor_scalar(
                out=act, in0=io129[0:BLOC, :], scalar1=endf, scalar2=None,
                op0=AL.is_le,
            )
            pact = pcb[0:T, 0, 192 : 192 + BLOC]
            ptrans(pact, act[:, 1:])
            nc.scalar.copy(out=actT, in_=pact)

            # y one-hots
            for b in range(BLOC):
                nc.vector.tensor_scalar(
                    out=ybig[:, b, :],
                    in0=io48[0:T, :],
                    scalar1=fidx[:, b : b + 1],
                    scalar2=actT[:, b : b + 1],
                    op0=AL.is_equal,
                    op1=AL.mult,
                )
            nc.sync.dma_start(out=y[:, :, :], in_=ybig)

    nc.finalize()
    _legalize_sync_waits(nc)
    return nc


def _legalize_sync_waits(nc):
    """This container's walrus accepts at most ONE sync wait per instruction.

    Split excess waits onto Drain instructions inserted just before the
    offending instruction (same engine, so the waits still complete before it
    issues; an idle-pipe Drain costs ~12ns).  Applied to the serialized BIR
    only -- CoreSim consumes the in-memory module and is unaffected.
    """
    import json as _json

    m = _json.loads(nc.to_json_bytes())
    for f in m["functions"]:
        for blk in f["blocks"]:
            out = []
            for ins in blk["instructions"]:
                si = ins.get("sync_info") or {}
                w = si.get("on_wait") or []
                if len(w) > 1:
                    for j, wx in enumerate(w[:-1]):
                        out.append(
                            {
                                "debug": ins.get("debug", 0),
                                "engine": ins["engine"],
                                "ins": [],
                                "outs": [],
                                "name": f"{ins['name']}-w{j}",
                                "opcode": "Drain",
                                "sync_info": {"on_update": [], "on_wait": [wx]},
                            }
                        )
                    si["on_wait"] = [w[-1]]
                out.append(ins)
            blk["instructions"] = out
    blob = _json.dumps(m).encode()
    nc.to_json_bytes = lambda: blob


def make_consts():
    f32 = np.float32
    c = np.zeros((128, CW), f32)
    c[0:L, 97] = NEG
    c[0, 97] = 0.0
    d0 = c[0:L, 97].copy()
    c[0:P2, 96] = np.concatenate([d0, d0])
    c[0:L, 98:227] = np.arange(T + 1, dtype=f32)[None, :]
    c[:, 227:275] = np.arange(L, dtype=f32)[None, :]
    return c


def make_in_maps(X, t_feats, e_feats):
    f32 = np.float32
    t_feats = np.asarray(t_feats, dtype=f32)
    e_feats = np.asarray(e_feats, dtype=f32)
    c = make_consts()
    c[0:P2, 0:L] = np.vstack([t_feats, t_feats])
    c[0:L, L : 2 * L] = t_feats.T

    idb = np.eye(128, dtype=f32)

    # e blob [v%128, kp, term, j, L] fp8: 4-term exact-to-~2^-16 split of e^T
    fp8 = mybir.dt.np(FP8)
    eTf = np.zeros((NK * 128, L), f32)
    eTf[:V] = np.ascontiguousarray(e_feats.T)
    terms = []
    t0 = eTf.astype(fp8)
    terms.append(t0)
    rs = (eTf - t0.astype(f32)) * 256.0
    for _ in range(NTERM - 1):
        t = rs.astype(fp8)
        terms.append(t)
        rs = rs - t.astype(f32)
    efm = np.ascontiguousarray(
        np.stack(terms, axis=1)              # [NK*128, NTERM, L]
        .reshape(KP, 2, 128, NTERM, L)       # [kp, j, p, term, L]
        .transpose(2, 0, 3, 1, 4)            # [p, kp, term, j, L]
    )

    # x blob per core [v%128, kp, j, (b t)] in fp8 (one-hot: exact)
    X = np.asarray(X)
    in_maps = []
    for ci in range(NCORES):
        Xc = np.zeros((BLOC, T, NK * 128), f32)
        Xc[:, :, :V] = X[ci * BLOC : (ci + 1) * BLOC]
        # [b, t, kp, j, p] -> [p, kp, j, b, t]
        xb = np.ascontiguousarray(
            Xc.reshape(BLOC, T, KP, 2, 128)
            .transpose(4, 2, 3, 0, 1)
            .reshape(128, KP, 2, BT)
        ).astype(fp8)
        in_maps.append({"x": xb, "eT": efm, "consts": c, "idb": idb})
    return in_maps


_NC = None


def _get_nc():
    global _NC
    if _NC is None:
        _NC = build_nc()
    return _NC


def kernel(X, t_feats, e_feats):
    in_maps = make_in_maps(X, t_feats, e_feats)
    nc = _get_nc()
    res = run_bass_kernel_spmd(nc, in_maps, list(range(NCORES)))
    out = np.concatenate(
        [res.results[ci]["y"].transpose(1, 0, 2) for ci in range(NCORES)], axis=0
    )
    return np.ascontiguousarray(out, dtype=np.float32)


# revision 42
# speedup vs baseline: 1.0120x; 1.0007x over previous
"""CRF Viterbi decode kernel for Trainium2 (Bass), data-parallel over batch.

Problem shapes (hardcoded): X [32,128,10000] f32 one-hot, t_feats [48,48],
e_feats [48,10000].  Output Y_hat [32,128,48] f32 one-hot.

Sharding: batch 32 -> 8 cores x 4.  t_feats / e_feats replicated.

Per-core pipeline (4 batch elems, labels on 48 partitions throughout):
  1. emissions em[l, (b,t)] = e_feats @ X^T: the host pre-transposes the
     one-hot X into an fp8 [v%128, kpair, j, (b t)] blob (0/1 exact in
     fp8) and splits e_feats^T into 4 exact fp8 terms (term 0 unscaled
     into PSUM bank A; terms 1-3 in 2^8-scaled space into bank B;
     combined as A + 2^-8 B).  Emissions are then 160 DoubleRow fp8
     matmuls (2 v-chunks per pass, 0.5 cyc/row) chasing the sliced X
     DMA -- no on-device transposes or staging.
  2. forward Viterbi: per pair of batch elems a PSUM-resident score
     tile psc[j, (sub i)] = t[i,j] + delta[i], updated per step with
     two 48-col f32 broadcast matmuls per elem: on-path bcast(m_{p-1})
     and off-path bcast(demf[p-3] - m_{p-2}) (2-step slack, built on
     the idle Pool engine).  One merged DVE tensor_reduce over both
     pairs' banks produces m_p for all 4 elems.  Periodic staggered
     full refreshes bound fp drift (exact f32 elsewhere).
  3. backward "end-anytime" scan, same structure with t^T, 56-stride
     segments whose permanent zero columns implement beta = max(0, .).
     Runs anti-phased with the forward scan: per ~1us step the two
     chains interleave PE broadcast clusters with DVE reduces.
  4. combine: y_n = onehot(argmax_j delta_n(j)+beta_n(j)) * [n <=
     end_n] via per-elem PE transposes + DVE max/max_index (first-index
     tie semantics match the reference argmax); the middle score
     columns are pre-summed during the scan loop.  Output is written
     label-minor [T, B, L] and unscrambled on the host.

Numerics notes for this stack: float32r matmuls round their inputs on
the NEFF/ucode path (CoreSim does not model it), so every scan matmul
stays plain f32; fp8/bf16 matmul inputs are exact by construction.
GPSIMD cannot touch PSUM, scalar_tensor_tensor does not exist on Pool,
and partition_all_reduce does not compile -- hence the DVE-only
reduces and Pool doing only small SBUF tensor_tensor work.
"""

import os
import sys

import numpy as np

for _p in ("/opt/trn_rl_repo",):
    if _p not in sys.path and os.path.isdir(_p):
        sys.path.insert(0, _p)

import concourse.bass as bass
import concourse.tile as tile
from concourse import mybir
from concourse.bass_utils import run_bass_kernel_spmd

F32 = mybir.dt.float32
F32R = mybir.dt.float32r
BF16 = mybir.dt.bfloat16
FP8 = mybir.dt.float8e4
AL = mybir.AluOpType
AX = mybir.AxisListType

B, T, V, L = 32, 128, 10000, 48
NCORES = 8
BLOC = B // NCORES          # 4 batch elems per core
NK = 80                     # V chunks of 128 (tail zero-padded; even for DoubleRow)
KP = NK // 2                # DoubleRow processes 2 chunks per matmul
NTERM = 4                   # fp8 e-term count (exact split of e_feats)
BT = BLOC * T               # 512 moving columns, b-major
NEG = -1.0e30
P2 = 2 * L                  # 96

# consts layout [128, CW] f32:
#  cols 0:48     tfS   [96,48]  vstack(t, t)        (fwd refresh lhsT)
#  cols 48:96    tbr   [48,48]  t.T                 (bwd refresh lhsT)
#  col  96       d0stack [96,1]
#  col  97       d0col   [48,1]
#  cols 98:227   iota129 [48,129]
#  cols 227:275  iota48  [128,48]
CW = 275

# fwd refresh schedule: always at p==2 (clears the +-1e30 d0 arithmetic),
# then every 16 steps staggered per pair.  bwd staggered likewise on t.
RF = 16


def _fwd_refresh(p, pair):
    if p == 2:
        return True
    return p > 2 and (p - 2 - 4 * pair) % RF == 0


def _bwd_refresh(t, pair):
    # phases 6/10 keep the last refreshes at t=118/122, avoiding a wasted
    # full rebuild on the backward scan's final steps
    if t == 0:
        return True
    return (t - 6 - 4 * pair) % RF == 0


def build_nc():
    nc = bass.Bass()

    x = nc.declare_dram_parameter("x", [128, KP, 2, BT], FP8, isOutput=False)
    eT = nc.declare_dram_parameter("eT", [128, KP, NTERM, 2, L], FP8, isOutput=False)
    consts = nc.declare_dram_parameter("consts", [128, CW], F32, isOutput=False)
    idb = nc.declare_dram_parameter("idb", [128, 128], F32, isOutput=False)
    y = nc.declare_dram_parameter("y", [T, BLOC, L], F32, isOutput=True)

    with tile.TileContext(nc) as tc:
        from contextlib import ExitStack

        with ExitStack() as ctx:
            cons = ctx.enter_context(tc.tile_pool(name="cons", bufs=1))
            pers = ctx.enter_context(tc.tile_pool(name="pers", bufs=1))
            ppem = ctx.enter_context(tc.tile_pool(name="ppem", bufs=1, space="PSUM"))
            ppfw = ctx.enter_context(tc.tile_pool(name="ppfw", bufs=1, space="PSUM"))
            ppbw = ctx.enter_context(tc.tile_pool(name="ppbw", bufs=1, space="PSUM"))
            ppcb = ctx.enter_context(tc.tile_pool(name="ppcb", bufs=1, space="PSUM"))

            # ---- constants (DMAs issued after the emission stream) ----
            cons_sb = cons.tile([128, CW], F32)
            d0col = cons_sb[0:L, 97:98]
            io129 = cons_sb[0:L, 98:227]
            io48 = cons_sb[:, 227:275]

            tfr = cons_sb[0:P2, 0:L]
            tbr = cons_sb[0:L, L : 2 * L]
            d0r = cons_sb[0:P2, 96:97]

            idb_sb = cons.tile([128, 128], F32)
            id96 = idb_sb[0:P2, 0:P2]
            id48 = idb_sb[0:L, 0:L]

            # ---- persistent state ----
            em48 = pers.tile([L, BLOC, T], F32, name="em48")
            demf = pers.tile([L, BLOC, T - 1], F32, name="demf")
            ndemf = pers.tile([L, BLOC, T - 1], F32, name="ndemf")
            mslab = pers.tile([L, BLOC, T + 1], F32, name="mslab")
            bslab = pers.tile([L, BLOC, T + 2], F32, name="bslab")
            ndm = pers.tile([L, 2, BLOC], F32, name="ndm")
            nbt = pers.tile([L, 2, BLOC], F32, name="nbt")
            dsl = pers.tile([L, BLOC, T + 1], F32, name="dsl")
            fsl = pers.tile([L, BLOC, T + 1], F32, name="fsl")
            fi8 = pers.tile([T, BLOC, 8], mybir.dt.uint32, name="fi8")
            fm8 = pers.tile([T, BLOC, 8], F32, name="fm8")
            fidx = pers.tile([T, BLOC], F32, name="fidx")
            nm = pers.tile([T, BLOC], F32, name="nm")
            nmb = pers.tile([BLOC, T + 1], F32, name="nmb")
            en8 = pers.tile([BLOC, 8], F32, name="en8")
            eni8 = pers.tile([BLOC, 8], mybir.dt.uint32, name="eni8")
            endf = pers.tile([BLOC, 1], F32, name="endf")
            act = pers.tile([BLOC, T + 1], F32, name="act")
            actT = pers.tile([T, BLOC], F32, name="actT")
            ybig = pers.tile([T, BLOC, L], F32, name="ybig")

            pscA = ppfw.tile([L, 2, 512], F32, name="pscA")
            psbA = ppbw.tile([L, 2, 512], F32, name="psbA")
            pem = ppem.tile([L, 2, 512], F32, name="pem")
            # bwd uses 49-wide segments; cols 48/97 stay 0 forever (the
            # max-with-zero floor).  beta_T = 0 (bslab col T).
            nc.vector.memset(psbA[:, :, 48:105:56], 0.0)
            nc.vector.memset(bslab[:, :, T : T + 1], 0.0)
            nc.vector.memset(bslab[:, :, 0:1], 0.0)

            def bmm(out, col, first=False, last=False, k96=False):
                """Broadcast col ([48,1] or [96,1]) along the free dim of out.

                lhsT/out are bitcast to f32r (same bits as f32 here) so the
                bf16 identity rhs keys the 1.0 cyc/row transpose path.
                """
                kk = P2 if k96 else L
                nc.tensor.matmul(
                    out,
                    col.broadcast_to([kk, L]),
                    id96 if k96 else id48,
                    start=first,
                    stop=last,
                    is_transpose=True,
                    skip_group_check=True,
                )

            def smm(out, lhsT, first=False):
                """Static 96-wide refresh matmul (t-term)."""
                nc.tensor.matmul(
                    out,
                    lhsT,
                    id96,
                    start=first,
                    stop=False,
                    is_transpose=True,
                    skip_group_check=True,
                )

            # ---- emissions: xt/ef live only in this phase; their pool is
            # closed afterwards so the beta slab can reuse the space ----
            epctx = tc.tile_pool(name="ep", bufs=1)
            ep = epctx.__enter__()
            ef = ep.tile([128, KP, NTERM, 2, L], FP8)
            xt = ep.tile([128, KP, 2, BT], FP8)
            KSL = [0, 5, 10, 15, 20, 25, 30, 35, KP]
            for s in range(len(KSL) - 1):
                k0, k1 = KSL[s], KSL[s + 1]
                nc.sync.dma_start(
                    out=ef[:, k0:k1, :, :, :], in_=eT[:, k0:k1, :, :, :]
                )
                nc.sync.dma_start(out=xt[:, k0:k1, :, :], in_=x[:, k0:k1, :, :])
                if s == 0:
                    # scan/combine constants ride behind the first slices
                    nc.sync.dma_start(out=cons_sb, in_=consts[:, :])
                    nc.sync.dma_start(out=idb_sb, in_=idb[:, :])
                for k in range(k0, k1):
                    for tm in range(NTERM):
                        # term 0 -> bank A (unscaled); terms 1-3 -> bank B
                        # (stored x256; the combine scales by 2^-8)
                        g = 0 if tm == 0 else 1
                        nc.tensor.matmul(
                            pem[:, g, :],
                            ef[:, k, tm, :, :],
                            xt[:, k, :, :],
                            start=(k == 0 and tm <= 1),
                            stop=(k == KP - 1 and tm in (0, NTERM - 1)),
                            perf_mode=mybir.MatmulPerfMode.DoubleRow,
                        )
            nc.scalar.copy(
                out=em48, in_=pem[:, 0, :].rearrange("p (b t) -> p b t", b=BLOC)
            )
            nc.vector.scalar_tensor_tensor(
                out=em48,
                in0=pem[:, 1, :].rearrange("p (b t) -> p b t", b=BLOC),
                scalar=1.0 / 256.0,
                in1=em48,
                op0=AL.mult,
                op1=AL.add,
            )
            nc.vector.tensor_sub(demf, em48[:, :, 1:], em48[:, :, 0 : T - 1])
            nc.gpsimd.tensor_sub(ndemf, em48[:, :, 0 : T - 1], em48[:, :, 1:])
            epctx.__exit__(None, None, None)  # xt/ef space no longer needed

            # ---- scans ----
            def fwd_step(t):
                p = t + 1
                # off-path bcast term (inputs >= 2 steps old), on Pool
                if p >= 3:
                    q = p % 2
                    nc.gpsimd.tensor_tensor(
                        ndm[:, q, :],
                        demf[:, :, p - 3],
                        mslab[:, :, p - 2],
                        op=AL.subtract,
                    )
                for pair in range(2):
                    ps = pscA[:, pair, :]
                    if p == 1:
                        smm(ps[:, 0:P2], tfr, first=True)
                        bmm(ps[:, 0:P2], d0r, k96=True)
                    elif _fwd_refresh(p, pair):
                        smm(ps[:, 0:P2], tfr, first=True)
                        for sub in range(2):
                            b = 2 * pair + sub
                            blk = ps[:, L * sub : L * sub + L]
                            bmm(blk, em48[:, b, p - 2 : p - 1])
                            bmm(blk, mslab[:, b, p - 1 : p])
                    else:
                        q = p % 2
                        for sub in range(2):
                            b = 2 * pair + sub
                            blk = ps[:, L * sub : L * sub + L]
                            bmm(blk, mslab[:, b, p - 1 : p])
                for pair in range(2):
                    if p == 1 or _fwd_refresh(p, pair):
                        continue
                    ps = pscA[:, pair, :]
                    q = p % 2
                    for sub in range(2):
                        b = 2 * pair + sub
                        blk = ps[:, L * sub : L * sub + L]
                        bmm(blk, ndm[:, q, b : b + 1])
                nc.vector.tensor_reduce(
                    out=mslab[:, :, p : p + 1],
                    in_=pscA[:, :, 0:P2].rearrange("p a (s i) -> p a s i", s=2),
                    axis=AX.X,
                    op=AL.max,
                )

            def bwd_step(t):
                pos = T - 1 - t
                if pos <= T - 2:
                    # nb_pos = -demf[pos] - beta_{pos+2} (off-path, Pool)
                    q = t % 2
                    nc.gpsimd.tensor_tensor(
                        nbt[:, q, :],
                        ndemf[:, :, pos],
                        bslab[:, :, pos + 2],
                        op=AL.subtract,
                    )
                for pair in range(2):
                    pb = psbA[:, pair, :]
                    rf = _bwd_refresh(t, pair)
                    for sub in range(2):
                        b = 2 * pair + sub
                        blk = pb[:, 56 * sub : 56 * sub + L]
                        if rf:
                            nc.tensor.matmul(
                                blk,
                                tbr,
                                id48,
                                start=(sub == 0),
                                stop=False,
                                is_transpose=True,
                                skip_group_check=True,
                            )
                            bmm(blk, em48[:, b, pos : pos + 1])
                            if t > 0:
                                bmm(blk, bslab[:, b, pos + 1 : pos + 2])
                        else:
                            bmm(blk, bslab[:, b, pos + 1 : pos + 2])
                for pair in range(2):
                    if _bwd_refresh(t, pair):
                        continue
                    pb = psbA[:, pair, :]
                    q = t % 2
                    for sub in range(2):
                        b = 2 * pair + sub
                        blk = pb[:, 56 * sub : 56 * sub + L]
                        bmm(blk, nbt[:, q, b : b + 1])
                nc.vector.tensor_reduce(
                    out=bslab[:, :, pos : pos + 1],
                    in_=psbA[:, :, 0:112].rearrange("p a (s i) -> p a s i", i=56)[
                        :, :, :, 0:49
                    ],
                    axis=AX.X,
                    op=AL.max,
                )

            # middle combine-sum columns [C0, C1) become ready mid-loop
            # (mslab fills forward, bslab backward: col n ready at
            # t = max(n-1, T-1-n))
            C0, C1 = 8, 122
            for t in range(T):
                fwd_step(t)
                if t < T - 1:  # beta_0 (t=127) is never consumed
                    bwd_step(t)
                if t == T - 8:
                    nc.gpsimd.tensor_add(
                        dsl[:, :, C0:C1], mslab[:, :, C0:C1], em48[:, :, C0 - 1 : C1 - 1]
                    )
                    nc.gpsimd.tensor_add(
                        fsl[:, :, C0:C1], dsl[:, :, C0:C1], bslab[:, :, C0:C1]
                    )

            # ---- combine ----
            id128f = idb_sb

            def ptrans(out_psum, in_sb):
                nc.tensor.matmul(
                    out_psum,
                    in_sb,
                    id128f[0 : in_sb.shape[0], 0 : in_sb.shape[0]],
                    start=True,
                    stop=True,
                    is_transpose=True,
                    skip_group_check=True,
                )

            # dsl: col0 = d0, cols 1: = m + em;  fsl = dsl + beta
            # (middle columns were computed during the scan loop)
            nc.vector.tensor_add(
                dsl[:, :, 1:C0], mslab[:, :, 1:C0], em48[:, :, 0 : C0 - 1]
            )
            nc.vector.tensor_add(
                dsl[:, :, C1:], mslab[:, :, C1:], em48[:, :, C1 - 1 :]
            )
            nc.vector.tensor_copy(dsl[:, :, 0:1], d0col.broadcast_to([L, BLOC, 1]))
            nc.vector.tensor_add(
                fsl[:, :, 0:C0], dsl[:, :, 0:C0], bslab[:, :, 0:C0]
            )
            nc.vector.tensor_add(
                fsl[:, :, C1:], dsl[:, :, C1:], bslab[:, :, C1 : T + 1]
            )

            # per-b transposes to [T, 48] (fresh PSUM banks) + max/argmax
            pcb = ppcb.tile([128, 2, 512], F32, name="pcb")
            for b in range(BLOC):
                ptd = pcb[:, 0, b * L : (b + 1) * L][0:T, :]
                ptf = pcb[:, 1, b * L : (b + 1) * L][0:T, :]
                ptrans(ptd, dsl[:, b, 1:])
                ptrans(ptf, fsl[:, b, 1:])
            nc.vector.tensor_reduce(
                out=nm,
                in_=pcb[0:T, 0, 0 : BLOC * L].rearrange("p (b l) -> p b l", b=BLOC),
                axis=AX.X,
                op=AL.max,
            )
            for b in range(BLOC):
                ptf = pcb[:, 1, b * L : (b + 1) * L][0:T, :]
                nc.vector.max(fm8[:, b, :], ptf)
                nc.vector.max_index(fi8[:, b, :], fm8[:, b, :], ptf)
            nc.vector.tensor_copy(fidx, fi8[:, :, 0])

            # end_n per b from n_maxs (col 0 of delta_full maxes to 0)
            pnm = pcb[0:BLOC, 1, 192 : 192 + T]
            ptrans(pnm, nm)
            nc.vector.memset(nmb[:, 0:1], 0.0)
            nc.scalar.copy(out=nmb[:, 1:], in_=pnm)
            nc.vector.max(en8, nmb)
            nc.vector.max_index(eni8, en8, nmb)
            nc.vector.tensor_copy(endf, eni8[:, 0:1])

            # active mask act[b, n] = (n <= end_n), transposed to [T, 4]
            nc.vector.tensor_scalar(
                out=act, in0=io129[0:BLOC, :], scalar1=endf, scalar2=None,
                op0=AL.is_le,
            )
            pact = pcb[0:T, 0, 192 : 192 + BLOC]
            ptrans(pact, act[:, 1:])
            nc.scalar.copy(out=actT, in_=pact)

            # y one-hots
            for b in range(BLOC):
                nc.vector.tensor_scalar(
                    out=ybig[:, b, :],
                    in0=io48[0:T, :],
                    scalar1=fidx[:, b : b + 1],
                    scalar2=actT[:, b : b + 1],
                    op0=AL.is_equal,
                    op1=AL.mult,
                )
            nc.sync.dma_start(out=y[:, :, :], in_=ybig)

    nc.finalize()
    _legalize_sync_waits(nc)
    return nc


def _legalize_sync_waits(nc):
    """This container's walrus accepts at most ONE sync wait per instruction.

    Split excess waits onto Drain instructions inserted just before the
    offending instruction (same engine, so the waits still complete before it
    issues; an idle-pipe Drain costs ~12ns).  Applied to the serialized BIR
    only -- CoreSim consumes the in-memory module and is unaffected.
    """
    import json as _json

    m = _json.loads(nc.to_json_bytes())
    for f in m["functions"]:
        for blk in f["blocks"]:
            out = []
            for ins in blk["instructions"]:
                si = ins.get("sync_info") or {}
                w = si.get("on_wait") or []
                if len(w) > 1:
                    for j, wx in enumerate(w[:-1]):
                        out.append(
                            {
                                "debug": ins.get("debug", 0),
                                "engine": ins["engine"],
                                "ins": [],
                                "outs": [],
                                "name": f"{ins['name']}-w{j}",
                                "opcode": "Drain",
                                "sync_info": {"on_update": [], "on_wait": [wx]},
                            }
                        )
                    si["on_wait"] = [w[-1]]
                out.append(ins)
            blk["instructions"] = out
    blob = _json.dumps(m).encode()
    nc.to_json_bytes = lambda: blob


def make_consts():
    f32 = np.float32
    c = np.zeros((128, CW), f32)
    c[0:L, 97] = NEG
    c[0, 97] = 0.0
    d0 = c[0:L, 97].copy()
    c[0:P2, 96] = np.concatenate([d0, d0])
    c[0:L, 98:227] = np.arange(T + 1, dtype=f32)[None, :]
    c[:, 227:275] = np.arange(L, dtype=f32)[None, :]
    return c


def make_in_maps(X, t_feats, e_feats):
    f32 = np.float32
    t_feats = np.asarray(t_feats, dtype=f32)
    e_feats = np.asarray(e_feats, dtype=f32)
    c = make_consts()
    c[0:P2, 0:L] = np.vstack([t_feats, t_feats])
    c[0:L, L : 2 * L] = t_feats.T

    idb = np.eye(128, dtype=f32)

    # e blob [v%128, kp, term, j, L] fp8: 4-term exact-to-~2^-16 split of e^T
    fp8 = mybir.dt.np(FP8)
    eTf = np.zeros((NK * 128, L), f32)
    eTf[:V] = np.ascontiguousarray(e_feats.T)
    terms = []
    t0 = eTf.astype(fp8)
    terms.append(t0)
    rs = (eTf - t0.astype(f32)) * 256.0
    for _ in range(NTERM - 1):
        t = rs.astype(fp8)
        terms.append(t)
        rs = rs - t.astype(f32)
    efm = np.ascontiguousarray(
        np.stack(terms, axis=1)              # [NK*128, NTERM, L]
        .reshape(KP, 2, 128, NTERM, L)       # [kp, j, p, term, L]
        .transpose(2, 0, 3, 1, 4)            # [p, kp, term, j, L]
    )

    # x blob per core [v%128, kp, j, (b t)] in fp8 (one-hot: exact)
    X = np.asarray(X)
    in_maps = []
    for ci in range(NCORES):
        Xc = np.zeros((BLOC, T, NK * 128), f32)
        Xc[:, :, :V] = X[ci * BLOC : (ci + 1) * BLOC]
        # [b, t, kp, j, p] -> [p, kp, j, b, t]
        xb = np.ascontiguousarray(
            Xc.reshape(BLOC, T, KP, 2, 128)
            .transpose(4, 2, 3, 0, 1)
            .reshape(128, KP, 2, BT)
        ).astype(fp8)
        in_maps.append({"x": xb, "eT": efm, "consts": c, "idb": idb})
    return in_maps


_NC = None


def _get_nc():
    global _NC
    if _NC is None:
        _NC = build_nc()
    return _NC


def kernel(X, t_feats, e_feats):
    in_maps = make_in_maps(X, t_feats, e_feats)
    nc = _get_nc()
    res = run_bass_kernel_spmd(nc, in_maps, list(range(NCORES)))
    out = np.concatenate(
        [res.results[ci]["y"].transpose(1, 0, 2) for ci in range(NCORES)], axis=0
    )
    return np.ascontiguousarray(out, dtype=np.float32)
